# revision 55
# baseline (speedup 1.0000x reference)
"""Trainium2 Bass kernel for the 3-scale anchor DetectionLoss (fast path).

Sharding: data-parallel over batch (16 samples -> 8 cores x 2 samples).
Each core computes the six partial accumulators for its 2 samples; the
host sums the per-core partials and applies the global normalizer.

Fast-path algorithm (per core), v2 (engine-rebalanced two-pass):
- Score proxy: for anchor A and box B, x = inter/(areaA+areaB+1e-9) is a
  strictly monotone transform of IOU per pair, so pos/neg thresholds and
  the per-anchor argmax all come from x. All 3 scales' x-scores come from
  one K=21 bf16 block-diagonal matmul per box on the PE into PSUM
  (host-precomputed rw'/c tables streamed per 4-box chunk).
- Pass 1: PE fills 4-box score chunks (double-buffered PSUM); DVE only
  max-reduces them into BESTX (the per-anchor best score).
- Pass 2: PE recomputes the identical scores and accumulates -BESTX on
  top via an fp32r identity matmul; ACT converts the residual into bf16
  winner masks (Relu(y*K + 2)); DVE only does the matched-content gather:
  ONE copy_predicated of 2 packed fp32 planes per box (bcx|bcy and
  lnwb|lnhb|label, 12/12 and 11/11/2-bit fields).
- Losses: packed content decoded with mod/subtract; SmoothL1 via the
  identity 0.5*m^2 - m + |d| (m=min(|d|,1), Square/Abs on ACT); pos/neg
  masks computed per-scale with fused count accum_out; softplus on ACT
  chain; hard-negative mining via threshold bisection (unchanged).
- No GPSIMD (it shares SBUF ports with DVE and poisons its throughput).

Generic fallback: if the anchors are not a consistent grid, fall back to
a numpy implementation.
"""

import numpy as np
import ml_dtypes
from contextlib import ExitStack

import concourse.bass as bass
import concourse.tile as tile
from concourse import bacc, mybir
from concourse import bass_utils
from concourse import bass_isa
from concourse.dve_spec import (Spec, Src0, Src1, C0, C2, Zero, One,
                                sq, maxx, minn, select, eq, lower)
from concourse.dve_spec import AluOp as DAlu
from concourse.dve_ops import DveOp, OPS, DveOpSpec, get_dve_sub_opcode
from concourse.dve_ops import has_src1 as _has_src1


def _register_dve_op(name, spec, subdim=False):
    """Register a custom DVE op at import time, self-pinning its uop sha."""
    import concourse.dve_ops as _dom
    for ex in OPS:
        if ex.name == name:
            return ex
    op = DveOp(name, spec, subdim=subdim, uops_sha={})
    OPS.append(op)
    row = _dom._CUSTOM_DVE_ROW_BASE + len(OPS) - 1
    assert row < 0x20, "custom DVE opcode rows exhausted"
    _dom._SUB_OPCODE_FOR_NAME[name] = row
    _dom.CUSTOM_DVE_SPECS[name] = spec
    for ver in ("v3", "v4"):
        tmp = DveOpSpec(name=name, opcode=get_dve_sub_opcode(name),
                        uops=lower(spec, ver=ver), rd1_en=_has_src1(spec))
        op.uops_sha[ver] = tmp.sha(ver)
    return op


def _ref_sl1f(in0, in1, s0, s1, imm2):
    d = in0.astype(np.float32) + in1
    ad = np.abs(d)
    m = np.minimum(ad, np.float32(1.0))
    return ((m - 1.0) ** 2 * imm2 + ad).astype(np.float32)


def _ref_selsum(in0, in1, s0, s1, imm2):
    b = np.where(in1 == s0, in0.astype(np.float32), 0.0).astype(np.float32)
    return b, b.reshape(b.shape[0], -1).sum(axis=-1, keepdims=True)


def _ref_neglf(in0, in1, s0, s1, imm2):
    return ((in0.astype(np.float32) + 1.0) * in1 - 1.0).astype(np.float32)


_D = Src0 + Src1
_AD = maxx(_D, Zero - _D)
ANT_SL1F = _register_dve_op("ANT_SL1F", Spec(
    body=sq(minn(_AD, One) - One) * C2 + _AD,
    reference=_ref_sl1f))
ANT_SELSUM = _register_dve_op("ANT_SELSUM", Spec(
    body=select(eq(Src1, C0), Src0, Zero),
    accum=DAlu.ADD,
    reference=_ref_selsum))
ANT_NEGLF = _register_dve_op("ANT_NEGLF", Spec(
    body=(Src0 + One) * Src1 - One,
    reference=_ref_neglf))

F32 = mybir.dt.float32
F32R = mybir.dt.float32r
I32 = mybir.dt.int32
U8 = mybir.dt.uint8
BF16 = mybir.dt.bfloat16
TAB_DT = BF16
Alu = mybir.AluOpType
Act = mybir.ActivationFunctionType
Red = bass_isa.ReduceOp

NCORES = 8
SPC = 2          # samples per core
NBOX = 40
P = 128
FCOL = 504
NITER = 5        # bisection iterations for top-k threshold
MINE_LO = 0.0    # negl = softplus(x)+1 in [1,~6.5] for neg anchors, else -1
MINE_HI = 10.0
MASK_K = 1.0e6   # winner-mask scale: band = 128/MASK_K in score units

# (H, W, HW, L, col_off) ; L = locations per partition
SCALES = [
    (128, 128, 16384, 128, 0),
    (64, 64, 4096, 32, 384),
    (32, 32, 1024, 8, 480),
]
SCOLS = ((0, 384), (384, 480), (480, 504))
THR_POS = float(np.float32(1.0 / 3.0))
THR_NEG = float(np.float32(3.0 / 13.0))

# content quantization: plane1 = qx*4096 + qy (11+12 bits),
# plane2 = qw*8192 + qh*4 + lab (11+11+2 bits). Fields extracted on
# device via round-to-nearest fp32->int32 conversion with a bias of
# -(0.5 - 2^-13) (floor emulation; high fields kept to 11 bits so the
# biased value stays exactly representable).
QX_S = 2.0           # px -> qx step 0.5
QY_S = 4.0           # px -> qy step 0.25
QLN_OFF = 2.9
QLN_S = 1000.0       # ln -> q step 1e-3
FLOOR_C = float(np.float32(0.5 - 2.0 ** -13))


# =====================================================================
# fast device body
# =====================================================================

def _build_fast(tc, aps):
    nc = tc.nc
    dve = nc.vector
    act = nc.scalar
    pe = nc.tensor

    pred_aps = [aps["pred0"], aps["pred1"], aps["pred2"]]

    with ExitStack() as ctx:
        pstat = ctx.enter_context(tc.tile_pool(name="stat", bufs=1))
        pwork = ctx.enter_context(tc.tile_pool(name="work", bufs=1))
        pscr = ctx.enter_context(tc.tile_pool(name="scr", bufs=1))
        pbit = ctx.enter_context(tc.tile_pool(name="bit", bufs=3))
        pbt = ctx.enter_context(tc.tile_pool(name="bt", bufs=3))

        # ---------------- static tiles (DMAs deferred: the score-table
        # stream must hit the DMA queue first so pass 1 starts early) ----
        ANCPK = pstat.tile([P, 4032], F32, tag="ancpk", name="ancpk")
        SCL4 = ANCPK[:, 0:2016]          # content dequant scales x|y|w|h
        OFF4 = ANCPK[:, 2016:4032]       # acx*rwa | acy*rha | lnwa-c | lnha-c
        IDM = pstat.tile([P, 128], F32R, tag="idm", name="idm")
        SMPK = pstat.tile([P, 2 * NBOX * SPC], F32, tag="smpk", name="smpk")

        def static_dma():
            # separate trigger engine -> separate DMA queue, so these bulk
            # loads don't head-block the per-chunk score-table stream
            nc.gpsimd.dma_start(IDM[:], aps["idm"])
            nc.gpsimd.dma_start(SMPK[:], aps["smpk"])
            nc.gpsimd.dma_start(ANCPK[:], aps["ancpk"])

        PREDB = [pstat.tile([P, 4032], F32, tag=f"pred{b}", name=f"pred{b}")
                 for b in range(SPC)]

        def pred_dma(b):
            for s, (H, W, HW, L, co) in enumerate(SCALES):
                for a in range(3):
                    s_v = pred_aps[s][b, a * 8:(a + 1) * 8].rearrange(
                        "f h w -> f (h w)").rearrange(
                        "f (p g) -> p f g", p=P)
                    d_v = PREDB[b][:].rearrange(
                        "p (f c) -> p f c", f=8)[:, :, co + a * L:
                                                 co + (a + 1) * L]
                    nc.gpsimd.dma_start(d_v, s_v)

        ONES128 = pstat.tile([P, 1], F32, tag="o128", name="o128")
        nc.gpsimd.memset(ONES128[:], 1.0)
        ALL1 = pstat.tile([P, 128], F32, tag="all1", name="all1")
        nc.gpsimd.memset(ALL1[:], 1.0)
        B128 = pstat.tile([P, 1], F32, tag="b128", name="b128")
        nc.gpsimd.memset(B128[:], 128.0)
        BN7 = pstat.tile([P, 1], F32, tag="bn7", name="bn7")
        nc.gpsimd.memset(BN7[:], -0.7071067811865476)
        ONES1 = pstat.tile([1, 128], F32, tag="o1", name="o1")
        nc.gpsimd.memset(ONES1[:], 1.0)

        # ---------------- persistent working tiles ----------------
        BESTX = pwork.tile([P, 1008], F32, tag="bestx", name="bestx")
        nc.gpsimd.memset(BESTX[:], 0.0)
        BESTYM = pwork.tile([P, 1008], F32R, tag="bestym", name="bestym")
        POSA = pwork.tile([P, 1008], F32, tag="posa", name="posa")
        NEGA = pwork.tile([P, 1008], F32, tag="nega", name="nega")
        NEGL = pwork.tile([P, 1008], F32, tag="negl", name="negl")
        # matched content: 2 packed planes, q-pitch 506 (shared by samples;
        # DVE program order serializes sample0 losses before sample1 gather)
        MQ2 = pwork.tile([P, 1012], F32, tag="mq2", name="mq2")
        MLAB = pwork.tile([P, FCOL], F32, tag="mlab", name="mlab")
        QI = pwork.tile([P, FCOL], I32, tag="qi", name="qi")
        LSE = pwork.tile([P, 1008], F32, tag="lse", name="lse")
        # partial accumulators: cols 0-5 obj/cls/loc per sample,
        # 6-11 npos(b,s), 12-17 nneg(b,s), 18-23 CE picked-logit sums (b,c)
        PARTALL = pwork.tile([P, 24], F32, tag="partall", name="partall")

        BIG = [pscr.tile([P, 4032], F32, tag=f"big{i}", name=f"big{i}")
               for i in range(3)]

        ppsum = None   # bound inside the psA pool scope below

        # ---------------- score chunk matmuls (4 boxes) ----------------
        def mm_chunk(PS, b, k, stop):
            twh = pbt.tile([21, 2528], TAB_DT, tag="twh", name="twh")
            nc.sync.dma_start(twh[:], aps["tabpk"][b, k])
            for slot in range(4):
                pe.matmul(PS[:, slot * 512:slot * 512 + FCOL],
                          twh[0:21, 2016 + slot * 128:
                              2016 + (slot + 1) * 128],
                          twh[0:21, slot * FCOL:(slot + 1) * FCOL],
                          start=True, stop=stop)

        # ---------------- pass 1: best score ----------------
        def pass1(b, mid_cb=None):
            bx = BESTX[:, b * FCOL:(b + 1) * FCOL]
            red = BIG[1][:, 0:FCOL]
            for k in range(10):
                if k == 1 and mid_cb is not None:
                    mid_cb()
                PS = ppsum.tile([P, 2048], F32, tag="ps", name="ps")
                mm_chunk(PS, b, k, stop=True)
                v = PS[:].rearrange("p (s c) -> p c s", s=4)[:, 0:FCOL, :]
                dve.tensor_reduce(red, v, mybir.AxisListType.X, Alu.max)
                dve.tensor_tensor(bx, bx, red, Alu.max)

        # ---------------- masks + per-scale counts + BESTYM ----------
        def masks(b):
            for s, (c0, c1) in enumerate(SCOLS):
                dve.tensor_scalar(
                    POSA[:, b * FCOL + c0:b * FCOL + c1],
                    BESTX[:, b * FCOL + c0:b * FCOL + c1],
                    THR_POS, 0.0, Alu.is_ge, Alu.add,
                    accum_out=PARTALL[:, 6 + 3 * b + s:7 + 3 * b + s])
                dve.tensor_scalar(
                    NEGA[:, b * FCOL + c0:b * FCOL + c1],
                    BESTX[:, b * FCOL + c0:b * FCOL + c1],
                    THR_NEG, 0.0, Alu.is_lt, Alu.add,
                    accum_out=PARTALL[:, 12 + 3 * b + s:13 + 3 * b + s])
            dve.tensor_scalar(BESTYM[:, b * FCOL:(b + 1) * FCOL],
                              BESTX[:, b * FCOL:(b + 1) * FCOL],
                              -1.0, None, Alu.mult)

        # ---------------- pass 2: winner masks + content gather -------
        def pass2(b, stage_cb=None):
            bymr = BESTYM[:, b * FCOL:(b + 1) * FCOL]
            idmr = IDM[:]
            cv = SMPK[:, 2 * NBOX * b:2 * NBOX * (b + 1)].rearrange(
                "p (q j) -> p q j", q=2)
            mqv = MQ2[:].rearrange("p (q c) -> p q c", q=2)[:, :, 0:FCOL]
            for k in range(10):
                PS = ppsum.tile([P, 2048], F32, tag="ps", name="ps")
                mm_chunk(PS, b, k, stop=False)
                for slot in range(4):
                    pe.matmul(PS[:, slot * 512:slot * 512 + FCOL],
                              idmr, bymr, start=False, stop=True)
                bt = pbit.tile([P, 4 * FCOL], U8, tag="bit", name="bit")
                btv = bt[:].rearrange("p (s c) -> p s c", s=4)
                psv = PS[:].rearrange("p (s c) -> p s c", s=4)[:, :, 0:FCOL]
                act.activation(btv, psv, Act.Relu, bias=B128[:],
                               scale=MASK_K)
                # one gather for the whole chunk: the out AP revisits the
                # same [P,2,504] region per slot (slot-major stream order),
                # so later boxes overwrite earlier ones like the per-box
                # sequence did
                dve.copy_predicated(
                    mqv.unsqueeze(1).broadcast_to([P, 4, 2, FCOL]),
                    btv.unsqueeze(2).broadcast_to([P, 4, 2, FCOL]),
                    cv[:, :, 4 * k:4 * k + 4].rearrange(
                        "p q j -> p j q").unsqueeze(3).broadcast_to(
                        [P, 4, 2, FCOL]))
                if stage_cb is not None:
                    stage_cb(k)

        # ---------------- early per-sample pieces (only need PREDB +
        # masks): obj BCE accumulation, NEGL for mining, and the CE
        # log-sum-exp — keeps ACT's exp/ln chain ahead of the mask stream.
        def objneg(b):
            posb = POSA[:, b * FCOL:(b + 1) * FCOL]
            negb = NEGA[:, b * FCOL:(b + 1) * FCOL]
            X = PREDB[b][:, 4 * FCOL:5 * FCOL]
            ax = BIG[1][:, 0:FCOL]
            ex = BIG[1][:, 504:1008]
            sp = BIG[1][:, 1008:1512]
            cacc = BIG[1][:, 1512:2016]
            act.activation(ax, X, Act.Abs)
            act.activation(ex, ax, Act.Exp, scale=-1.0)
            act.activation(ax, ex, Act.Ln, bias=1.0)
            dve.scalar_tensor_tensor(sp, X, 0.0, ax,
                                     Alu.max, Alu.add)
            dve.tensor_tensor(ex, sp, X, Alu.subtract)
            dve.scalar_tensor_tensor(cacc, ex, 0.0, posb,
                                     Alu.add, Alu.mult,
                                     accum_out=PARTALL[:, 3 * b:3 * b + 1])
            nb = NEGL[:, b * FCOL:(b + 1) * FCOL]
            dve._custom_dve(ANT_NEGLF, out=nb, in0=sp, in1=negb)
            # CE log-sum-exp (kept per sample in its own slot)
            CL0 = PREDB[b][:, 5 * FCOL:6 * FCOL]
            CL1 = PREDB[b][:, 6 * FCOL:7 * FCOL]
            CL2 = PREDB[b][:, 7 * FCOL:8 * FCOL]
            lse = LSE[:, b * FCOL:(b + 1) * FCOL]
            e1 = BIG[1][:, 0:FCOL]
            act.activation(lse, CL0, Act.Exp)
            act.activation(e1, CL1, Act.Exp)
            dve.tensor_tensor(lse, lse, e1, Alu.add)
            act.activation(e1, CL2, Act.Exp)
            dve.tensor_tensor(lse, lse, e1, Alu.add)
            act.activation(lse, lse, Act.Ln)

        # ---------------- per-sample losses ----------------
        def losses(b):
            posb = POSA[:, b * FCOL:(b + 1) * FCOL]
            g1 = MQ2[:, 0:FCOL]
            g2 = MQ2[:, 506:506 + FCOL]

            # ----- decode packed content -----
            # Integer fields extracted with fused round-to-int outputs:
            # floor(t) = round_to_int(t - (0.5 - 2^-13)); high fields are
            # 11-bit so the biased value stays exactly representable.
            CONT = BIG[0][:, 0:2016]
            CONTI = CONT.bitcast(I32)
            qxi = CONTI[:, 0:504]
            qyi = CONTI[:, 504:1008]
            qwi = CONTI[:, 1008:1512]
            qhi = CONTI[:, 1512:2016]
            rem = QI[:]
            dve.tensor_scalar(qxi, g1, 2.0 ** -12, -FLOOR_C,
                              Alu.mult, Alu.add)
            dve.scalar_tensor_tensor(qyi, qxi, -4096.0, g1,
                                     Alu.mult, Alu.add)
            dve.tensor_scalar(qwi, g2, 2.0 ** -13, -FLOOR_C,
                              Alu.mult, Alu.add)
            dve.scalar_tensor_tensor(rem, qwi, -8192.0, g2,
                                     Alu.mult, Alu.add)
            # qh = floor(rem / 4); lab = rem - 4*qh (lab in {1,2,3} so the
            # fractional part is in {.25,.5,.75}; bias 0.5 keeps round exact)
            dve.tensor_scalar(qhi, rem, 0.25, -0.5, Alu.mult, Alu.add)
            dve.scalar_tensor_tensor(MLAB[:], qhi, -4.0, rem,
                                     Alu.mult, Alu.add)

            # ----- loc (SmoothL1) -----
            # sl1 = 0.5*min(|d|,1)^2 - min(|d|,1) + |d|
            #     = 0.5*(m-1)^2 + |d| - 0.5, with the -0.5 folded into the
            #       masked accumulation below (scalar -2.0 over 4 planes)
            #       and the rest fused into one custom DVE op (ANT_SL1F).
            T1 = BIG[2][:, 0:2016]
            W = BIG[2][:, 2016:4032]
            dve.tensor_tensor(T1, CONTI, SCL4, Alu.mult)
            dve.tensor_tensor(W, PREDB[b][:, 0:2016], T1, Alu.subtract)
            SL = CONT  # reuse
            dve._custom_dve(ANT_SL1F, out=SL, in0=W, in1=OFF4, imm2=0.5)
            s2 = BIG[1][:, 2520:3528]
            dve.tensor_tensor(s2, SL[:, 0:1008], SL[:, 1008:2016], Alu.add)
            sl = BIG[1][:, 3528:4032]
            dve.tensor_tensor(sl, s2[:, 0:504], s2[:, 504:1008], Alu.add)
            cacc = BIG[1][:, 0:FCOL]
            dve.scalar_tensor_tensor(cacc, sl, -2.0, posb,
                                     Alu.add, Alu.mult,
                                     accum_out=PARTALL[:, 3 * b + 2:
                                                       3 * b + 3])

            # ----- CE: sum_pos lse accumulated positively; the picked
            # class logit accumulated per class into cols 18-23 (subtracted
            # in the final combine) via the custom select-eq-sum op. -----
            CL0 = PREDB[b][:, 5 * FCOL:6 * FCOL]
            CL1 = PREDB[b][:, 6 * FCOL:7 * FCOL]
            CL2 = PREDB[b][:, 7 * FCOL:8 * FCOL]
            mlp = BIG[0][:, 2016:2520]
            selscr = BIG[0][:, 2520:3024]
            lse = LSE[:, b * FCOL:(b + 1) * FCOL]
            dve.scalar_tensor_tensor(cacc, lse, 0.0, posb,
                                     Alu.add, Alu.mult,
                                     accum_out=PARTALL[:, 3 * b + 1:
                                                       3 * b + 2])
            dve.tensor_tensor(mlp, MLAB[:], posb, Alu.mult)
            for c, CLp in enumerate((CL0, CL1, CL2)):
                dve._custom_dve(
                    ANT_SELSUM, out=selscr, in0=CLp, in1=mlp,
                    s0=float(c + 1),
                    accum_out=PARTALL[:, 18 + 3 * b + c:19 + 3 * b + c])

        # ================= hard-negative mining =================
        # Bisection with replicated [P,6] state, interleaved through the
        # pass-2 emission. Per-segment counts come from ACT via the Sign
        # trick: count(>thr) = (sum sign(negl-thr) + Ntot)/2 (non-neg
        # anchors hold negl=-1 and contribute -1 each, absorbed by Ntot).
        # Cross-partition sums via GPSIMD partition_all_reduce (no PSUM).
        gp = nc.gpsimd
        t6 = lambda n: pwork.tile([P, 6], F32, tag=n, name=n)
        K6 = t6("k6")
        K2W = t6("k2w")
        WTOT = t6("wtot")
        LO = t6("lo6")
        HI = t6("hi6")
        MID = t6("mid6")
        NMID = t6("nmid6")
        GTK = t6("gtk6")
        DD = t6("dd6")
        CNTA = t6("cnta")
        CNTR = t6("cntr")
        SG = t6("sg6")
        KK = t6("kk6")
        NP12R = pwork.tile([P, 12], F32, tag="np12r", name="np12r")
        CG12 = pwork.tile([P, 12], F32, tag="cg12", name="cg12")
        CGR = pwork.tile([P, 12], F32, tag="cgr", name="cgr")
        MSCR = pwork.tile([P, 384], F32, tag="mscr", name="mscr")
        SEGS = [(b, c0, c1) for b in range(SPC) for (c0, c1) in SCOLS]

        def mine_prep():
            for i, (b, c0, c1) in enumerate(SEGS):
                nc.gpsimd.memset(WTOT[:, i:i + 1], float(P * (c1 - c0)))
            gp.partition_all_reduce(NP12R[:], PARTALL[:, 6:18], P, Red.add)
            dve.tensor_scalar(K6[:], NP12R[:, 0:6], 1.0, 3.0,
                              Alu.max, Alu.mult)
            dve.tensor_tensor(K6[:], K6[:], NP12R[:, 6:12], Alu.min)
            dve.tensor_scalar(K2W[:], K6[:], 2.0, None, Alu.mult)
            dve.tensor_tensor(K2W[:], K2W[:], WTOT[:], Alu.subtract)
            dve.memset(LO[:], MINE_LO)
            dve.memset(HI[:], MINE_HI)

        def mine_stageA(it):
            dve.tensor_tensor(MID[:], LO[:], HI[:], Alu.add)
            dve.tensor_scalar(MID[:], MID[:], 0.5, None, Alu.mult)
            dve.tensor_scalar(NMID[:], MID[:], -1.0, None, Alu.mult)
            for i, (b, c0, c1) in enumerate(SEGS):
                act.activation(MSCR[:, 0:c1 - c0],
                               NEGL[:, b * FCOL + c0:b * FCOL + c1],
                               Act.Sign, bias=NMID[:, i:i + 1],
                               accum_out=CNTA[:, i:i + 1])
            gp.partition_all_reduce(CNTR[:], CNTA[:], P, Red.add)

        def mine_stageB(it):
            dve.tensor_tensor(GTK[:], CNTR[:], K2W[:], Alu.is_gt)
            dve.tensor_tensor(DD[:], MID[:], LO[:], Alu.subtract)
            dve.tensor_tensor(DD[:], GTK[:], DD[:], Alu.mult)
            dve.tensor_tensor(LO[:], LO[:], DD[:], Alu.add)
            dve.tensor_tensor(DD[:], HI[:], MID[:], Alu.subtract)
            dve.tensor_tensor(DD[:], GTK[:], DD[:], Alu.mult)
            dve.tensor_tensor(HI[:], MID[:], DD[:], Alu.add)

        def mine_final_acts():
            dve.tensor_scalar(NMID[:], HI[:], -1.0, None, Alu.mult)
            for i, (b, c0, c1) in enumerate(SEGS):
                sl_ = NEGL[:, b * FCOL + c0:b * FCOL + c1]
                act.activation(MSCR[:, 0:c1 - c0], sl_, Act.Sign,
                               bias=NMID[:, i:i + 1],
                               accum_out=CG12[:, i:i + 1])
                act.activation(MSCR[:, 0:c1 - c0], sl_, Act.Relu,
                               bias=NMID[:, i:i + 1],
                               accum_out=CG12[:, 6 + i:7 + i])
            gp.partition_all_reduce(CGR[:], CG12[:], P, Red.add)

        def mine_final_kk():
            # cnt(>HI) = (sgn_sum + Ntot)/2 ; S(>HI) = relu_sum + cnt*HI
            dve.tensor_tensor(CNTR[:], CGR[:, 0:6], WTOT[:], Alu.add)
            dve.tensor_scalar(CNTR[:], CNTR[:], 0.5, None, Alu.mult)
            dve.tensor_tensor(SG[:], CNTR[:], HI[:], Alu.mult)
            dve.tensor_tensor(SG[:], SG[:], CGR[:, 6:12], Alu.add)
            # stragglers lie in (LO, HI]; estimate by the interval midpoint
            dve.tensor_tensor(MID[:], LO[:], HI[:], Alu.add)
            dve.tensor_scalar(MID[:], MID[:], 0.5, None, Alu.mult)
            dve.tensor_tensor(KK[:], K6[:], CNTR[:], Alu.subtract)
            dve.tensor_tensor(KK[:], KK[:], MID[:], Alu.mult)
            dve.tensor_tensor(KK[:], KK[:], SG[:], Alu.add)

        def mine_stage0(k):
            # A(i) at even chunks, B(i) at the following even chunk
            if k % 2 == 0:
                it = k // 2
                if it > 0:
                    mine_stageB(it - 1)
                if it < NITER:
                    mine_stageA(it)

        def mine_stage1(k):
            if k == 0 and NITER >= 5:
                mine_stageB(NITER - 1)
                mine_final_acts()
            elif k == 2:
                mine_final_kk()

        # ================= emit the pipeline =================
        with tc.psum_pool(name="psA", bufs=2) as psA:
            ppsum = psA
            static_dma()
            pass1(0, mid_cb=lambda: pred_dma(0))
            masks(0)
            objneg(0)
            pass1(1, mid_cb=lambda: pred_dma(1))
            masks(1)
            objneg(1)
            mine_prep()
            pass2(0, stage_cb=mine_stage0)
            losses(0)
            pass2(1, stage_cb=mine_stage1)
            losses(1)

        # ================= final cross-partition sums =================
        ppsB = ctx.enter_context(tc.psum_pool(name="psB", bufs=1))
        SUMP = ppsB.tile([P, 24], F32, tag="sump", name="sump")
        pe.matmul(SUMP[:], ALL1[:], PARTALL[:])
        SUMR = pwork.tile([P, 24], F32, tag="sumr", name="sumr")
        dve.tensor_copy(SUMR[:], SUMP[:])
        np6 = SUMR[:, 6:12]

        # ---------------- final combine + store ----------------
        OUTT = pwork.tile([1, 8], F32, tag="outt", name="outt")
        s1 = pwork.tile([1, 1], F32, tag="s1", name="s1")
        # obj = objp0 + objp1 + sum(KK)
        dve.tensor_reduce(s1[:], KK[0:1, :], mybir.AxisListType.X, Alu.add)
        dve.tensor_tensor(OUTT[:, 0:1], SUMR[0:1, 0:1], SUMR[0:1, 3:4],
                          Alu.add)
        dve.tensor_tensor(OUTT[:, 0:1], OUTT[:, 0:1], s1[:], Alu.add)
        dve.tensor_tensor(OUTT[:, 1:2], SUMR[0:1, 1:2], SUMR[0:1, 4:5],
                          Alu.add)
        dve.tensor_reduce(s1[:], SUMR[0:1, 18:24], mybir.AxisListType.X,
                          Alu.add)
        dve.tensor_tensor(OUTT[:, 1:2], OUTT[:, 1:2], s1[:], Alu.subtract)
        dve.tensor_tensor(OUTT[:, 2:3], SUMR[0:1, 2:3], SUMR[0:1, 5:6],
                          Alu.add)
        dve.tensor_reduce(s1[:], np6[0:1, :], mybir.AxisListType.X, Alu.add)
        dve.tensor_copy(OUTT[:, 3:4], s1[:])
        dve.tensor_reduce(s1[:], K6[0:1, :], mybir.AxisListType.X, Alu.add)
        dve.tensor_copy(OUTT[:, 4:5], s1[:])
        nc.sync.dma_start(aps["out"], OUTT[:])


# =====================================================================
# host-side grid extraction + packing
# =====================================================================

_HOSTC = {}


def _extract_grid(anchors):
    """anchors: list of 3 [A,4] arrays. Returns dict or None if not grid."""
    out = {"X1": [], "X2": [], "Y1": [], "Y2": []}
    for s, (H, W, HW, L, co) in enumerate(SCALES):
        a4 = np.asarray(anchors[s], np.float32).reshape(H, W, 3, 4)
        x1 = a4[0, :, :, 0]          # [W,3]
        x2 = a4[0, :, :, 2]
        y1 = a4[:, 0, :, 1]          # [H,3]
        y2 = a4[:, 0, :, 3]
        if not (np.array_equal(a4[:, :, :, 0], np.broadcast_to(x1, (H, W, 3)))
                and np.array_equal(a4[:, :, :, 2],
                                   np.broadcast_to(x2, (H, W, 3)))
                and np.array_equal(a4[:, :, :, 1],
                                   np.broadcast_to(y1[:, None], (H, W, 3)))
                and np.array_equal(a4[:, :, :, 3],
                                   np.broadcast_to(y2[:, None], (H, W, 3)))):
            return None
        out["X1"].append(x1.T.copy())   # [3, W]
        out["X2"].append(x2.T.copy())
        out["Y1"].append(y1.T.copy())
        out["Y2"].append(y2.T.copy())
    return out


def _anchor_layout(vals, s):
    """[A] per-anchor values -> [128, 3L] tile block (col = a*L + g)."""
    H, W, HW, L, co = SCALES[s]
    return np.ascontiguousarray(
        vals.reshape(P, L, 3).transpose(0, 2, 1).reshape(P, 3 * L))


def _host_static(anchors):
    """Sample-independent packs: ancpk [128,4160] (SCL4|OFF4|Id), grid
    tables, area0 [3,3] (scale, a)."""
    key = "static"
    if key in _HOSTC:
        return _HOSTC[key]
    grid = _extract_grid(anchors)
    if grid is None:
        _HOSTC[key] = None
        return None
    scl4 = np.zeros((P, 2016), np.float32)
    off4 = np.zeros((P, 2016), np.float32)
    area0 = np.zeros((3, 3), np.float32)
    sx = np.float32(1.0 / QX_S)
    sy = np.float32(1.0 / QY_S)
    sw = np.float32(1.0 / QLN_S)
    sh = np.float32(1.0 / QLN_S)
    for s, (H, W, HW, L, co) in enumerate(SCALES):
        a4 = np.asarray(anchors[s], np.float32)
        aw = a4[:, 2] - a4[:, 0]
        ah = a4[:, 3] - a4[:, 1]
        acx = a4[:, 0] + np.float32(0.5) * aw
        acy = a4[:, 1] + np.float32(0.5) * ah
        area0[s] = (aw * ah)[0:3]
        rwa = (np.float32(1.0) / aw).astype(np.float32)
        rha = (np.float32(1.0) / ah).astype(np.float32)
        lnwa = np.log(aw).astype(np.float32)
        lnha = np.log(ah).astype(np.float32)
        scl = {0: sx * rwa, 1: sy * rha,
               2: np.full_like(rwa, sw), 3: np.full_like(rwa, sh)}
        off = {0: acx * rwa, 1: acy * rha,
               2: lnwa - np.float32(QLN_OFF), 3: lnha - np.float32(QLN_OFF)}
        for q in range(4):
            scl4[:, q * FCOL + co:q * FCOL + co + 3 * L] = _anchor_layout(
                scl[q].astype(np.float32), s)
            off4[:, q * FCOL + co:q * FCOL + co + 3 * L] = _anchor_layout(
                off[q].astype(np.float32), s)
    ancpk = np.concatenate([scl4, off4], axis=1)

    res = {"ancpk": np.ascontiguousarray(ancpk),
           "idm": np.eye(P, dtype=np.float32),
           "grid": grid, "area0": area0}
    _HOSTC[key] = res
    return res


def _host_percore(boxes_c, labels_c, static):
    """boxes_c [2,40,4], labels_c [2,40] -> tabpk [2,10,21,2528],
    smpk [128, 2*40*SPC] (packed content planes)."""
    area0 = static["area0"]
    grid = static["grid"]
    tabpk = np.zeros((SPC, 10, 21, 2528), np.float32)
    smpk = np.zeros((P, 2 * NBOX * SPC), np.float32)

    def tables(s, bx):
        """rw' [3,40,W], rh [3,40,H] for scale s (f32 stepwise)."""
        X1, X2 = grid["X1"][s], grid["X2"][s]
        Y1, Y2 = grid["Y1"][s], grid["Y2"][s]
        wb = bx[:, 2] - bx[:, 0]
        hb = bx[:, 3] - bx[:, 1]
        ab = wb * hb
        cs = (area0[s][:, None] + ab[None, :]).astype(np.float32) \
            + np.float32(1e-9)
        rcs = (np.float32(1.0) / cs).astype(np.float32)
        rw = np.minimum(X2[:, None, :], bx[None, :, 2:3]) \
            - np.maximum(X1[:, None, :], bx[None, :, 0:1])
        rw = np.maximum(rw, np.float32(0.0)) * rcs[:, :, None]
        rh = np.minimum(Y2[:, None, :], bx[None, :, 3:4]) \
            - np.maximum(Y1[:, None, :], bx[None, :, 1:2])
        rh = np.maximum(rh, np.float32(0.0))
        return rw.astype(np.float32), rh.astype(np.float32)

    pidx = np.arange(P)
    for b in range(SPC):
        bx = np.asarray(boxes_c[b], np.float32)
        wb = bx[:, 2] - bx[:, 0]
        hb = bx[:, 3] - bx[:, 1]
        rw0, rh0 = tables(0, bx)
        rw1, rh1 = tables(1, bx)
        rw2, rh2 = tables(2, bx)
        # scale1: lhsT[(a,par), p] = rh1[a,j,p//2]*(p%2==par); rhs
        # [(a,par),(a',g)] = delta(a,a')*rw1'[a,j,par*32+g]
        lh1 = np.zeros((NBOX, 6, 128), np.float32)
        rs1 = np.zeros((NBOX, 6, 96), np.float32)
        for a in range(3):
            for par in range(2):
                kk = a * 2 + par
                lh1[:, kk, :] = rh1[a][:, pidx // 2] * (pidx % 2 == par)
                rs1[:, kk, a * 32:(a + 1) * 32] = \
                    rw1[a][:, par * 32:(par + 1) * 32]
        lh2 = np.zeros((NBOX, 12, 128), np.float32)
        rs2 = np.zeros((NBOX, 12, 24), np.float32)
        for a in range(3):
            for qd in range(4):
                kk = a * 4 + qd
                lh2[:, kk, :] = rh2[a][:, pidx // 4] * (pidx % 4 == qd)
                rs2[:, kk, a * 8:(a + 1) * 8] = \
                    rw2[a][:, qd * 8:(qd + 1) * 8]
        for k in range(10):
            for slot in range(4):
                j = 4 * k + slot
                c0 = slot * FCOL
                for a in range(3):
                    tabpk[b, k, a, c0 + a * 128:c0 + (a + 1) * 128] = \
                        rw0[a, j]
                tabpk[b, k, 3:9, c0 + 384:c0 + 480] = rs1[j]
                tabpk[b, k, 9:21, c0 + 480:c0 + 504] = rs2[j]
                l0 = 2016 + slot * 128
                tabpk[b, k, 0:3, l0:l0 + 128] = rh0[:, j]
                tabpk[b, k, 3:9, l0:l0 + 128] = lh1[j]
                tabpk[b, k, 9:21, l0:l0 + 128] = lh2[j]
        # packed content planes: plane0 = qx*4096+qy, plane1 = qw*8192+qh*4+lab
        gcx = bx[:, 0] + np.float32(0.5) * wb
        gcy = bx[:, 1] + np.float32(0.5) * hb
        qx = np.clip(np.round(gcx * QX_S), 0, 2047).astype(np.float64)
        qy = np.clip(np.round(gcy * QY_S), 0, 4095).astype(np.float64)
        qw = np.clip(np.round((np.log(wb) - QLN_OFF) * QLN_S),
                     0, 2047).astype(np.float64)
        qh = np.clip(np.round((np.log(hb) - QLN_OFF) * QLN_S),
                     0, 2047).astype(np.float64)
        lab = np.asarray(labels_c[b], np.float64)
        p0 = (qx * 4096.0 + qy).astype(np.float32)
        p1 = (qw * 8192.0 + qh * 4.0 + lab).astype(np.float32)
        base = 2 * NBOX * b
        smpk[:, base:base + NBOX] = p0[None, :]
        smpk[:, base + NBOX:base + 2 * NBOX] = p1[None, :]
    return tabpk, smpk


# =====================================================================
# compile + run
# =====================================================================

_CACHE = {}


def _get_compiled_fast():
    if "fast" in _CACHE:
        return _CACHE["fast"]
    nc = bacc.Bacc("TRN2", target_bir_lowering=False, debug=False)
    aps = {
        "pred0": nc.dram_tensor("pred0", [SPC, 24, 128, 128], F32,
                                kind="ExternalInput").ap(),
        "pred1": nc.dram_tensor("pred1", [SPC, 24, 64, 64], F32,
                                kind="ExternalInput").ap(),
        "pred2": nc.dram_tensor("pred2", [SPC, 24, 32, 32], F32,
                                kind="ExternalInput").ap(),
        "ancpk": nc.dram_tensor("ancpk", [P, 4032], F32,
                                kind="ExternalInput").ap(),
        "idm": nc.dram_tensor("idm", [P, 128], F32R,
                              kind="ExternalInput").ap(),
        "tabpk": nc.dram_tensor("tabpk", [SPC, 10, 21, 2528], TAB_DT,
                                kind="ExternalInput").ap(),
        "smpk": nc.dram_tensor("smpk", [P, 2 * NBOX * SPC], F32,
                               kind="ExternalInput").ap(),
        "out": nc.dram_tensor("out", [1, 8], F32, kind="ExternalOutput").ap(),
    }
    with tile.TileContext(nc) as tc:
        _build_fast(tc, aps)
    nc.compile()
    _CACHE["fast"] = (nc, None)
    return _CACHE["fast"]


def _kernel_numpy(pred0, pred1, pred2, anchors0, anchors1, anchors2,
                  boxes, labels):
    """Self-contained numpy fallback (only for non-grid anchors)."""
    def softplus(x):
        return np.log1p(np.exp(-np.abs(x))) + np.maximum(x, 0.0)

    tot = np.zeros(5, np.float64)
    for pred, anc in ((pred0, anchors0), (pred1, anchors1),
                      (pred2, anchors2)):
        B, ch, H, W = pred.shape
        p = pred.transpose(0, 2, 3, 1).reshape(B, H * W * 3, 8)
        anc = np.asarray(anc, np.float64)
        aa = (anc[:, 2] - anc[:, 0]) * (anc[:, 3] - anc[:, 1])
        for b in range(B):
            bx = np.asarray(boxes[b], np.float64)
            ab = (bx[:, 2] - bx[:, 0]) * (bx[:, 3] - bx[:, 1])
            lt = np.maximum(anc[:, None, :2], bx[None, :, :2])
            rb = np.minimum(anc[:, None, 2:], bx[None, :, 2:])
            wh = np.clip(rb - lt, 0.0, None)
            inter = wh[..., 0] * wh[..., 1]
            iou = inter / (aa[:, None] + ab[None, :] - inter + 1e-9)
            best = iou.max(1)
            bidx = iou.argmax(1)
            pos = best >= 0.5
            neg = best < 0.3
            x = p[b, :, 4]
            oall = softplus(x) - x * pos
            npos = int(pos.sum())
            k = int(min(neg.sum(), 3 * max(npos, 1)))
            nl = np.where(neg, softplus(x), -1.0)
            order = np.argsort(-nl, kind="stable")
            sel = np.zeros(len(x), bool)
            sel[order[:k]] = True
            sel &= neg
            tot[0] += oall[pos | sel].sum()
            logit = p[b, :, 5:]
            m = logit.max(-1, keepdims=True)
            lse = np.log(np.exp(logit - m).sum(-1)) + m[:, 0]
            tgt = np.clip(labels[b][bidx] - 1, 0, 2)
            ce = lse - np.take_along_axis(logit, tgt[:, None], 1)[:, 0]
            tot[1] += ce[pos].sum()
            mb = bx[bidx]
            aw = anc[:, 2] - anc[:, 0]
            ah = anc[:, 3] - anc[:, 1]
            enc = np.stack([
                (0.5 * (mb[:, 0] + mb[:, 2]) - (anc[:, 0] + 0.5 * aw)) / aw,
                (0.5 * (mb[:, 1] + mb[:, 3]) - (anc[:, 1] + 0.5 * ah)) / ah,
                np.log((mb[:, 2] - mb[:, 0]) / aw),
                np.log((mb[:, 3] - mb[:, 1]) / ah)], -1)
            d = np.abs(p[b, :, :4] - enc)
            sl1 = np.where(d < 1.0, 0.5 * d * d, d - 0.5).sum(-1)
            tot[2] += sl1[pos].sum()
            tot[3] += npos
            tot[4] += int(sel.sum())
    norm = np.float32(max(tot[3], 1.0))
    lo = np.float32(tot[0] / norm)
    lc = np.float32(tot[1] / norm)
    ll = np.float32(tot[2] / norm)
    return (lo, lc, ll, np.float32(lo + lc + 2.0 * ll),
            np.float32(tot[3]), np.float32(tot[4]))


def kernel(pred0, pred1, pred2, anchors0, anchors1, anchors2, boxes, labels,
           _want_results=False, _trace=False):
    static = _host_static([anchors0, anchors1, anchors2])
    if static is None:   # pragma: no cover
        out = _kernel_numpy(pred0, pred1, pred2, anchors0, anchors1,
                            anchors2, boxes, labels)
        out = tuple(np.asarray(v, np.float32) for v in out)
        return (out, None) if _want_results else out
    nc, _ = _get_compiled_fast()
    in_maps = []
    for c in range(NCORES):
        sl = slice(c * SPC, (c + 1) * SPC)
        tabpk, smpk = _host_percore(boxes[sl], labels[sl], static)
        tabpk = tabpk.astype(ml_dtypes.bfloat16)
        in_maps.append({
            "pred0": np.ascontiguousarray(pred0[sl], np.float32),
            "pred1": np.ascontiguousarray(pred1[sl], np.float32),
            "pred2": np.ascontiguousarray(pred2[sl], np.float32),
            "ancpk": static["ancpk"],
            "idm": static["idm"],
            "tabpk": np.ascontiguousarray(tabpk),
            "smpk": np.ascontiguousarray(smpk),
        })
    res = bass_utils.run_bass_kernel_spmd(
        nc, in_maps, core_ids=list(range(NCORES)), trace=_trace)
    parts = np.stack([res.results[c]["out"][0] for c in range(NCORES)])
    tot = parts.sum(axis=0, dtype=np.float64).astype(np.float32)
    tot_obj, tot_cls, tot_loc, tot_pos, tot_neg = tot[:5]
    norm = np.float32(max(tot_pos, np.float32(1.0)))
    lo = np.float32(tot_obj / norm)
    lc = np.float32(tot_cls / norm)
    ll = np.float32(tot_loc / norm)
    ltot = np.float32(lo + lc + np.float32(2.0) * ll)
    out = (lo, lc, ll, ltot, np.float32(tot_pos), np.float32(tot_neg))
    out = tuple(np.asarray(v, np.float32) for v in out)
    if _want_results:
        return out, res
    return out


# revision 63
# speedup vs baseline: 1.1280x; 1.1280x over previous
"""Trainium2 Bass kernel for the 3-scale anchor DetectionLoss (fast path).

Sharding: data-parallel over batch (16 samples -> 8 cores x 2 samples).
Each core computes the six partial accumulators for its 2 samples; the
host sums the per-core partials and applies the global normalizer.

Fast-path algorithm (per core), v2 (engine-rebalanced two-pass):
- Score proxy: for anchor A and box B, x = inter/(areaA+areaB+1e-9) is a
  strictly monotone transform of IOU per pair, so pos/neg thresholds and
  the per-anchor argmax all come from x. All 3 scales' x-scores come from
  one K=21 bf16 block-diagonal matmul per box on the PE into PSUM
  (host-precomputed rw'/c tables streamed per 4-box chunk).
- Pass 1: PE fills 4-box score chunks (double-buffered PSUM); DVE only
  max-reduces them into BESTX (the per-anchor best score).
- Pass 2: PE recomputes the identical scores and accumulates -BESTX on
  top via an fp32r identity matmul; ACT converts the residual into bf16
  winner masks (Relu(y*K + 2)); DVE only does the matched-content gather:
  ONE copy_predicated of 2 packed fp32 planes per box (bcx|bcy and
  lnwb|lnhb|label, 12/12 and 11/11/2-bit fields).
- Losses: packed content decoded with mod/subtract; SmoothL1 via the
  identity 0.5*m^2 - m + |d| (m=min(|d|,1), Square/Abs on ACT); pos/neg
  masks computed per-scale with fused count accum_out; softplus on ACT
  chain; hard-negative mining via threshold bisection (unchanged).
- No GPSIMD (it shares SBUF ports with DVE and poisons its throughput).

Generic fallback: if the anchors are not a consistent grid, fall back to
a numpy implementation.
"""

import numpy as np
import ml_dtypes
from contextlib import ExitStack

import concourse.bass as bass
import concourse.tile as tile
from concourse import bacc, mybir
from concourse import bass_utils
from concourse import bass_isa
from concourse.dve_spec import (Spec, Src0, Src1, C0, C2, Zero, One,
                                sq, maxx, minn, select, eq, lower)
from concourse.dve_spec import AluOp as DAlu
from concourse.dve_ops import DveOp, OPS, DveOpSpec, get_dve_sub_opcode
from concourse.dve_ops import has_src1 as _has_src1


def _register_dve_op(name, spec, subdim=False):
    """Register a custom DVE op at import time, self-pinning its uop sha."""
    import concourse.dve_ops as _dom
    for ex in OPS:
        if ex.name == name:
            return ex
    op = DveOp(name, spec, subdim=subdim, uops_sha={})
    OPS.append(op)
    row = _dom._CUSTOM_DVE_ROW_BASE + len(OPS) - 1
    assert row < 0x20, "custom DVE opcode rows exhausted"
    _dom._SUB_OPCODE_FOR_NAME[name] = row
    _dom.CUSTOM_DVE_SPECS[name] = spec
    for ver in ("v3", "v4"):
        tmp = DveOpSpec(name=name, opcode=get_dve_sub_opcode(name),
                        uops=lower(spec, ver=ver), rd1_en=_has_src1(spec))
        op.uops_sha[ver] = tmp.sha(ver)
    return op


def _ref_sl1f(in0, in1, s0, s1, imm2):
    d = in0.astype(np.float32) + in1
    ad = np.abs(d)
    m = np.minimum(ad, np.float32(1.0))
    return ((m - 1.0) ** 2 * imm2 + ad).astype(np.float32)


def _ref_selsum(in0, in1, s0, s1, imm2):
    b = np.where(in1 == s0, in0.astype(np.float32), 0.0).astype(np.float32)
    return b, b.reshape(b.shape[0], -1).sum(axis=-1, keepdims=True)


def _ref_neglf(in0, in1, s0, s1, imm2):
    return ((in0.astype(np.float32) + 1.0) * in1 - 1.0).astype(np.float32)


_D = Src0 + Src1
_AD = maxx(_D, Zero - _D)
ANT_SL1F = _register_dve_op("ANT_SL1F", Spec(
    body=sq(minn(_AD, One) - One) * C2 + _AD,
    reference=_ref_sl1f))
ANT_SELSUM = _register_dve_op("ANT_SELSUM", Spec(
    body=select(eq(Src1, C0), Src0, Zero),
    accum=DAlu.ADD,
    reference=_ref_selsum))
ANT_NEGLF = _register_dve_op("ANT_NEGLF", Spec(
    body=(Src0 + One) * Src1 - One,
    reference=_ref_neglf))

F32 = mybir.dt.float32
F32R = mybir.dt.float32r
I32 = mybir.dt.int32
U8 = mybir.dt.uint8
BF16 = mybir.dt.bfloat16
TAB_DT = BF16
Alu = mybir.AluOpType
Act = mybir.ActivationFunctionType
Red = bass_isa.ReduceOp

NCORES = 8
SPC = 2          # samples per core
NBOX = 40
P = 128
FCOL = 504
NITER = 5        # bisection iterations for top-k threshold
MINE_LO = 0.0    # negl = softplus(x)+1 in [1,~6.5] for neg anchors, else -1
MINE_HI = 10.0
MASK_K = 1.0e6   # winner-mask scale: band = 128/MASK_K in score units

# (H, W, HW, L, col_off) ; L = locations per partition
SCALES = [
    (128, 128, 16384, 128, 0),
    (64, 64, 4096, 32, 384),
    (32, 32, 1024, 8, 480),
]
SCOLS = ((0, 384), (384, 480), (480, 504))
THR_POS = float(np.float32(1.0 / 3.0))
THR_NEG = float(np.float32(3.0 / 13.0))

# content quantization, ONE packed plane per box:
#   v = lab*2^22 + qx*2^15 + qy*2^8 + qw*2^4 + qh   (2|7|7|4|4 bits)
# lab is extracted exactly with two is_ge ops; the coordinate fields via
# floor(t) = round_to_int(t - (0.5 - 2^-(bl+1))) which stays exactly
# representable for each level's bit split.
QXY_S = 1.0 / 8.0    # px -> qx step 8
QLN_OFF = 2.9
QLN_S = 7.0          # ln -> q step 1/7
LAB_B = float(2.0 ** 22)


def _floor_c(bl):
    return float(np.float32(0.5 - 2.0 ** -(bl + 1)))


# =====================================================================
# fast device body
# =====================================================================

def _build_fast(tc, aps):
    nc = tc.nc
    dve = nc.vector
    act = nc.scalar
    pe = nc.tensor

    pred_aps = [aps["pred0"], aps["pred1"], aps["pred2"]]

    with ExitStack() as ctx:
        pstat = ctx.enter_context(tc.tile_pool(name="stat", bufs=1))
        pwork = ctx.enter_context(tc.tile_pool(name="work", bufs=1))
        pscr = ctx.enter_context(tc.tile_pool(name="scr", bufs=1))
        pbit = ctx.enter_context(tc.tile_pool(name="bit", bufs=3))
        pbt = ctx.enter_context(tc.tile_pool(name="bt", bufs=3))

        # ---------------- static tiles (DMAs deferred: the score-table
        # stream must hit the DMA queue first so pass 1 starts early) ----
        ANCPK = pstat.tile([P, 4032], F32, tag="ancpk", name="ancpk")
        SCL4 = ANCPK[:, 0:2016]          # content dequant scales x|y|w|h
        OFF4 = ANCPK[:, 2016:4032]       # acx*rwa | acy*rha | lnwa-c | lnha-c
        IDM = pstat.tile([P, 128], F32R, tag="idm", name="idm")
        SMPK = pstat.tile([P, NBOX * SPC], F32, tag="smpk", name="smpk")

        def static_dma():
            # separate trigger engine -> separate DMA queue, so these bulk
            # loads don't head-block the per-chunk score-table stream
            nc.gpsimd.dma_start(IDM[:], aps["idm"])
            nc.gpsimd.dma_start(SMPK[:], aps["smpk"])
            nc.gpsimd.dma_start(ANCPK[:], aps["ancpk"])

        PREDB = [pstat.tile([P, 4032], F32, tag=f"pred{b}", name=f"pred{b}")
                 for b in range(SPC)]

        def pred_dma(b):
            for s, (H, W, HW, L, co) in enumerate(SCALES):
                for a in range(3):
                    s_v = pred_aps[s][b, a * 8:(a + 1) * 8].rearrange(
                        "f h w -> f (h w)").rearrange(
                        "f (p g) -> p f g", p=P)
                    d_v = PREDB[b][:].rearrange(
                        "p (f c) -> p f c", f=8)[:, :, co + a * L:
                                                 co + (a + 1) * L]
                    nc.gpsimd.dma_start(d_v, s_v)

        ONES128 = pstat.tile([P, 1], F32, tag="o128", name="o128")
        nc.gpsimd.memset(ONES128[:], 1.0)
        ALL1 = pstat.tile([P, 128], F32, tag="all1", name="all1")
        nc.gpsimd.memset(ALL1[:], 1.0)
        B128 = pstat.tile([P, 1], F32, tag="b128", name="b128")
        nc.gpsimd.memset(B128[:], 128.0)
        BN7 = pstat.tile([P, 1], F32, tag="bn7", name="bn7")
        nc.gpsimd.memset(BN7[:], -0.7071067811865476)
        ONES1 = pstat.tile([1, 128], F32, tag="o1", name="o1")
        nc.gpsimd.memset(ONES1[:], 1.0)

        # ---------------- persistent working tiles ----------------
        BESTX = pwork.tile([P, 1008], F32, tag="bestx", name="bestx")
        nc.gpsimd.memset(BESTX[:], 0.0)
        BESTYM = pwork.tile([P, 1008], F32R, tag="bestym", name="bestym")
        POSA = pwork.tile([P, 1008], F32, tag="posa", name="posa")
        NEGA = pwork.tile([P, 1008], F32, tag="nega", name="nega")
        NEGL = pwork.tile([P, 1008], F32, tag="negl", name="negl")
        # matched content: 1 packed plane (shared by samples; DVE program
        # order serializes sample0 losses before sample1 gather)
        MQ2 = pwork.tile([P, FCOL], F32, tag="mq2", name="mq2")
        MLAB = pwork.tile([P, FCOL], F32, tag="mlab", name="mlab")
        QI = pwork.tile([P, FCOL], I32, tag="qi", name="qi")
        LSE = pwork.tile([P, 1008], F32, tag="lse", name="lse")
        # partial accumulators: cols 0-5 obj/cls/loc per sample,
        # 6-11 npos(b,s), 12-17 nneg(b,s), 18-23 CE picked-logit sums (b,c)
        PARTALL = pwork.tile([P, 24], F32, tag="partall", name="partall")

        BIG = [pscr.tile([P, 4032], F32, tag=f"big{i}", name=f"big{i}")
               for i in range(3)]

        ppsum = None   # bound inside the psA pool scope below

        # ---------------- score chunk matmuls (4 boxes) ----------------
        def mm_chunk(PS, b, k, stop):
            twh = pbt.tile([21, 2528], TAB_DT, tag="twh", name="twh")
            nc.sync.dma_start(twh[:], aps["tabpk"][b, k])
            for slot in range(4):
                pe.matmul(PS[:, slot * 512:slot * 512 + FCOL],
                          twh[0:21, 2016 + slot * 128:
                              2016 + (slot + 1) * 128],
                          twh[0:21, slot * FCOL:(slot + 1) * FCOL],
                          start=True, stop=stop)

        # ---------------- pass 1: best score ----------------
        def pass1(b, mid_cb=None):
            bx = BESTX[:, b * FCOL:(b + 1) * FCOL]
            red = BIG[1][:, 0:FCOL]
            for k in range(10):
                if k == 1 and mid_cb is not None:
                    mid_cb()
                PS = ppsum.tile([P, 2048], F32, tag="ps", name="ps")
                mm_chunk(PS, b, k, stop=True)
                v = PS[:].rearrange("p (s c) -> p c s", s=4)[:, 0:FCOL, :]
                dve.tensor_reduce(red, v, mybir.AxisListType.X, Alu.max)
                dve.tensor_tensor(bx, bx, red, Alu.max)

        # ---------------- masks + per-scale counts + BESTYM ----------
        def masks(b):
            for s, (c0, c1) in enumerate(SCOLS):
                dve.tensor_scalar(
                    POSA[:, b * FCOL + c0:b * FCOL + c1],
                    BESTX[:, b * FCOL + c0:b * FCOL + c1],
                    THR_POS, 0.0, Alu.is_ge, Alu.add,
                    accum_out=PARTALL[:, 6 + 3 * b + s:7 + 3 * b + s])
                dve.tensor_scalar(
                    NEGA[:, b * FCOL + c0:b * FCOL + c1],
                    BESTX[:, b * FCOL + c0:b * FCOL + c1],
                    THR_NEG, 0.0, Alu.is_lt, Alu.add,
                    accum_out=PARTALL[:, 12 + 3 * b + s:13 + 3 * b + s])
            dve.tensor_scalar(BESTYM[:, b * FCOL:(b + 1) * FCOL],
                              BESTX[:, b * FCOL:(b + 1) * FCOL],
                              -1.0, None, Alu.mult)

        # ---------------- pass 2: winner masks + content gather -------
        def pass2(b, stage_cb=None):
            bymr = BESTYM[:, b * FCOL:(b + 1) * FCOL]
            idmr = IDM[:]
            cv = SMPK[:, NBOX * b:NBOX * (b + 1)]
            for k in range(10):
                PS = ppsum.tile([P, 2048], F32, tag="ps", name="ps")
                mm_chunk(PS, b, k, stop=False)
                for slot in range(4):
                    pe.matmul(PS[:, slot * 512:slot * 512 + FCOL],
                              idmr, bymr, start=False, stop=True)
                bt = pbit.tile([P, 4 * FCOL], U8, tag="bit", name="bit")
                btv = bt[:].rearrange("p (s c) -> p s c", s=4)
                psv = PS[:].rearrange("p (s c) -> p s c", s=4)[:, :, 0:FCOL]
                act.activation(btv, psv, Act.Relu, bias=B128[:],
                               scale=MASK_K)
                # one gather for the whole chunk: the out AP revisits the
                # same [P,504] region per slot (slot-major stream order),
                # so later boxes overwrite earlier ones like the per-box
                # sequence did
                dve.copy_predicated(
                    MQ2[:].unsqueeze(1).broadcast_to([P, 4, FCOL]),
                    btv,
                    cv[:, 4 * k:4 * k + 4].unsqueeze(2).broadcast_to(
                        [P, 4, FCOL]))
                if stage_cb is not None:
                    stage_cb(k)

        # ---------------- early per-sample pieces (only need PREDB +
        # masks): obj BCE accumulation, NEGL for mining, and the CE
        # log-sum-exp — keeps ACT's exp/ln chain ahead of the mask stream.
        def objneg(b):
            posb = POSA[:, b * FCOL:(b + 1) * FCOL]
            negb = NEGA[:, b * FCOL:(b + 1) * FCOL]
            X = PREDB[b][:, 4 * FCOL:5 * FCOL]
            ax = BIG[1][:, 0:FCOL]
            ex = BIG[1][:, 504:1008]
            sp = BIG[1][:, 1008:1512]
            cacc = BIG[1][:, 1512:2016]
            act.activation(ax, X, Act.Abs)
            act.activation(ex, ax, Act.Exp, scale=-1.0)
            act.activation(ax, ex, Act.Ln, bias=1.0)
            dve.scalar_tensor_tensor(sp, X, 0.0, ax,
                                     Alu.max, Alu.add)
            dve.tensor_tensor(ex, sp, X, Alu.subtract)
            dve.scalar_tensor_tensor(cacc, ex, 0.0, posb,
                                     Alu.add, Alu.mult,
                                     accum_out=PARTALL[:, 3 * b:3 * b + 1])
            nb = NEGL[:, b * FCOL:(b + 1) * FCOL]
            dve._custom_dve(ANT_NEGLF, out=nb, in0=sp, in1=negb)
            # CE log-sum-exp (kept per sample in its own slot)
            CL0 = PREDB[b][:, 5 * FCOL:6 * FCOL]
            CL1 = PREDB[b][:, 6 * FCOL:7 * FCOL]
            CL2 = PREDB[b][:, 7 * FCOL:8 * FCOL]
            lse = LSE[:, b * FCOL:(b + 1) * FCOL]
            e1 = BIG[1][:, 0:FCOL]
            act.activation(lse, CL0, Act.Exp)
            act.activation(e1, CL1, Act.Exp)
            dve.tensor_tensor(lse, lse, e1, Alu.add)
            act.activation(e1, CL2, Act.Exp)
            dve.tensor_tensor(lse, lse, e1, Alu.add)
            act.activation(lse, lse, Act.Ln)

        # ---------------- per-sample losses ----------------
        def losses(b):
            posb = POSA[:, b * FCOL:(b + 1) * FCOL]
            g1 = MQ2[:, 0:FCOL]

            # ----- decode packed content -----
            # lab exactly via two is_ge thresholds; coordinate fields via
            # fused round-to-int floor extraction per level.
            CONT = BIG[0][:, 0:2016]
            CONTI = CONT.bitcast(I32)
            qxi = CONTI[:, 0:504]
            qyi = CONTI[:, 504:1008]
            qwi = CONTI[:, 1008:1512]
            qhi = CONTI[:, 1512:2016]
            t2 = BIG[1][:, 0:FCOL]
            rem = QI[:]
            dve.tensor_scalar(MLAB[:], g1, 2.0 * LAB_B, None, Alu.is_ge)
            dve.tensor_scalar(t2, g1, 3.0 * LAB_B, None, Alu.is_ge)
            dve.scalar_tensor_tensor(MLAB[:], MLAB[:], 1.0, t2,
                                     Alu.add, Alu.add)
            dve.scalar_tensor_tensor(rem, MLAB[:], -LAB_B, g1,
                                     Alu.mult, Alu.add)
            dve.tensor_scalar(qxi, rem, 2.0 ** -15, -_floor_c(15),
                              Alu.mult, Alu.add)
            dve.scalar_tensor_tensor(rem, qxi, -float(2 ** 15), rem,
                                     Alu.mult, Alu.add)
            dve.tensor_scalar(qyi, rem, 2.0 ** -8, -_floor_c(8),
                              Alu.mult, Alu.add)
            dve.scalar_tensor_tensor(rem, qyi, -256.0, rem,
                                     Alu.mult, Alu.add)
            dve.tensor_scalar(qwi, rem, 2.0 ** -4, -_floor_c(4),
                              Alu.mult, Alu.add)
            dve.scalar_tensor_tensor(qhi, qwi, -16.0, rem,
                                     Alu.mult, Alu.add)

            # ----- loc (SmoothL1) -----
            # sl1 = 0.5*min(|d|,1)^2 - min(|d|,1) + |d|
            #     = 0.5*(m-1)^2 + |d| - 0.5, with the -0.5 folded into the
            #       masked accumulation below (scalar -2.0 over 4 planes)
            #       and the rest fused into one custom DVE op (ANT_SL1F).
            T1 = BIG[2][:, 0:2016]
            W = BIG[2][:, 2016:4032]
            dve.tensor_tensor(T1, CONTI, SCL4, Alu.mult)
            dve.tensor_tensor(W, PREDB[b][:, 0:2016], T1, Alu.subtract)
            SL = CONT  # reuse
            dve._custom_dve(ANT_SL1F, out=SL, in0=W, in1=OFF4, imm2=0.5)
            s2 = BIG[1][:, 2520:3528]
            dve.tensor_tensor(s2, SL[:, 0:1008], SL[:, 1008:2016], Alu.add)
            sl = BIG[1][:, 3528:4032]
            dve.tensor_tensor(sl, s2[:, 0:504], s2[:, 504:1008], Alu.add)
            cacc = BIG[1][:, 0:FCOL]
            dve.scalar_tensor_tensor(cacc, sl, -2.0, posb,
                                     Alu.add, Alu.mult,
                                     accum_out=PARTALL[:, 3 * b + 2:
                                                       3 * b + 3])

            # ----- CE: sum_pos lse accumulated positively; the picked
            # class logit accumulated per class into cols 18-23 (subtracted
            # in the final combine) via the custom select-eq-sum op. -----
            CL0 = PREDB[b][:, 5 * FCOL:6 * FCOL]
            CL1 = PREDB[b][:, 6 * FCOL:7 * FCOL]
            CL2 = PREDB[b][:, 7 * FCOL:8 * FCOL]
            mlp = BIG[0][:, 2016:2520]
            selscr = BIG[0][:, 2520:3024]
            lse = LSE[:, b * FCOL:(b + 1) * FCOL]
            dve.scalar_tensor_tensor(cacc, lse, 0.0, posb,
                                     Alu.add, Alu.mult,
                                     accum_out=PARTALL[:, 3 * b + 1:
                                                       3 * b + 2])
            dve.tensor_tensor(mlp, MLAB[:], posb, Alu.mult)
            for c, CLp in enumerate((CL0, CL1, CL2)):
                dve._custom_dve(
                    ANT_SELSUM, out=selscr, in0=CLp, in1=mlp,
                    s0=float(c + 1),
                    accum_out=PARTALL[:, 18 + 3 * b + c:19 + 3 * b + c])

        # ================= hard-negative mining =================
        # Bisection with replicated [P,6] state, interleaved through the
        # pass-2 emission. Per-segment counts come from ACT via the Sign
        # trick: count(>thr) = (sum sign(negl-thr) + Ntot)/2 (non-neg
        # anchors hold negl=-1 and contribute -1 each, absorbed by Ntot).
        # Cross-partition sums via GPSIMD partition_all_reduce (no PSUM).
        gp = nc.gpsimd
        t6 = lambda n: pwork.tile([P, 6], F32, tag=n, name=n)
        K6 = t6("k6")
        K2W = t6("k2w")
        WTOT = t6("wtot")
        LO = t6("lo6")
        HI = t6("hi6")
        MID = t6("mid6")
        NMID = t6("nmid6")
        GTK = t6("gtk6")
        DD = t6("dd6")
        CNTA = t6("cnta")
        CNTR = t6("cntr")
        SG = t6("sg6")
        KK = t6("kk6")
        NP12R = pwork.tile([P, 12], F32, tag="np12r", name="np12r")
        CG12 = pwork.tile([P, 12], F32, tag="cg12", name="cg12")
        CGR = pwork.tile([P, 12], F32, tag="cgr", name="cgr")
        MSCR = pwork.tile([P, 384], F32, tag="mscr", name="mscr")
        SEGS = [(b, c0, c1) for b in range(SPC) for (c0, c1) in SCOLS]

        def mine_prep():
            for i, (b, c0, c1) in enumerate(SEGS):
                nc.gpsimd.memset(WTOT[:, i:i + 1], float(P * (c1 - c0)))
            gp.partition_all_reduce(NP12R[:], PARTALL[:, 6:18], P, Red.add)
            dve.tensor_scalar(K6[:], NP12R[:, 0:6], 1.0, 3.0,
                              Alu.max, Alu.mult)
            dve.tensor_tensor(K6[:], K6[:], NP12R[:, 6:12], Alu.min)
            dve.tensor_scalar(K2W[:], K6[:], 2.0, None, Alu.mult)
            dve.tensor_tensor(K2W[:], K2W[:], WTOT[:], Alu.subtract)
            dve.memset(LO[:], MINE_LO)
            dve.memset(HI[:], MINE_HI)

        def mine_stageA(it):
            dve.tensor_tensor(MID[:], LO[:], HI[:], Alu.add)
            dve.tensor_scalar(MID[:], MID[:], 0.5, None, Alu.mult)
            dve.tensor_scalar(NMID[:], MID[:], -1.0, None, Alu.mult)
            for i, (b, c0, c1) in enumerate(SEGS):
                act.activation(MSCR[:, 0:c1 - c0],
                               NEGL[:, b * FCOL + c0:b * FCOL + c1],
                               Act.Sign, bias=NMID[:, i:i + 1],
                               accum_out=CNTA[:, i:i + 1])
            gp.partition_all_reduce(CNTR[:], CNTA[:], P, Red.add)

        def mine_stageB(it):
            dve.tensor_tensor(GTK[:], CNTR[:], K2W[:], Alu.is_gt)
            dve.tensor_tensor(DD[:], MID[:], LO[:], Alu.subtract)
            dve.tensor_tensor(DD[:], GTK[:], DD[:], Alu.mult)
            dve.tensor_tensor(LO[:], LO[:], DD[:], Alu.add)
            dve.tensor_tensor(DD[:], HI[:], MID[:], Alu.subtract)
            dve.tensor_tensor(DD[:], GTK[:], DD[:], Alu.mult)
            dve.tensor_tensor(HI[:], MID[:], DD[:], Alu.add)

        def mine_final_acts():
            dve.tensor_scalar(NMID[:], HI[:], -1.0, None, Alu.mult)
            for i, (b, c0, c1) in enumerate(SEGS):
                sl_ = NEGL[:, b * FCOL + c0:b * FCOL + c1]
                act.activation(MSCR[:, 0:c1 - c0], sl_, Act.Sign,
                               bias=NMID[:, i:i + 1],
                               accum_out=CG12[:, i:i + 1])
                act.activation(MSCR[:, 0:c1 - c0], sl_, Act.Relu,
                               bias=NMID[:, i:i + 1],
                               accum_out=CG12[:, 6 + i:7 + i])
            gp.partition_all_reduce(CGR[:], CG12[:], P, Red.add)

        def mine_final_kk():
            # cnt(>HI) = (sgn_sum + Ntot)/2 ; S(>HI) = relu_sum + cnt*HI
            dve.tensor_tensor(CNTR[:], CGR[:, 0:6], WTOT[:], Alu.add)
            dve.tensor_scalar(CNTR[:], CNTR[:], 0.5, None, Alu.mult)
            dve.tensor_tensor(SG[:], CNTR[:], HI[:], Alu.mult)
            dve.tensor_tensor(SG[:], SG[:], CGR[:, 6:12], Alu.add)
            # stragglers lie in (LO, HI]; estimate by the interval midpoint
            dve.tensor_tensor(MID[:], LO[:], HI[:], Alu.add)
            dve.tensor_scalar(MID[:], MID[:], 0.5, None, Alu.mult)
            dve.tensor_tensor(KK[:], K6[:], CNTR[:], Alu.subtract)
            dve.tensor_tensor(KK[:], KK[:], MID[:], Alu.mult)
            dve.tensor_tensor(KK[:], KK[:], SG[:], Alu.add)

        def mine_stage0(k):
            # A(i) at even chunks, B(i) at the following even chunk
            if k % 2 == 0:
                it = k // 2
                if it > 0:
                    mine_stageB(it - 1)
                if it < NITER:
                    mine_stageA(it)

        def mine_stage1(k):
            if k == 0 and NITER >= 5:
                mine_stageB(NITER - 1)
                mine_final_acts()
            elif k == 2:
                mine_final_kk()

        # ================= emit the pipeline =================
        with tc.psum_pool(name="psA", bufs=2) as psA:
            ppsum = psA
            static_dma()
            pass1(0, mid_cb=lambda: pred_dma(0))
            masks(0)
            objneg(0)
            pass1(1, mid_cb=lambda: pred_dma(1))
            masks(1)
            objneg(1)
            mine_prep()
            pass2(0, stage_cb=mine_stage0)
            losses(0)
            pass2(1, stage_cb=mine_stage1)
            losses(1)

        # ================= final cross-partition sums =================
        ppsB = ctx.enter_context(tc.psum_pool(name="psB", bufs=1))
        SUMP = ppsB.tile([P, 24], F32, tag="sump", name="sump")
        pe.matmul(SUMP[:], ALL1[:], PARTALL[:])
        SUMR = pwork.tile([P, 24], F32, tag="sumr", name="sumr")
        dve.tensor_copy(SUMR[:], SUMP[:])
        np6 = SUMR[:, 6:12]

        # ---------------- final combine + store ----------------
        OUTT = pwork.tile([1, 8], F32, tag="outt", name="outt")
        s1 = pwork.tile([1, 1], F32, tag="s1", name="s1")
        # obj = objp0 + objp1 + sum(KK)
        dve.tensor_reduce(s1[:], KK[0:1, :], mybir.AxisListType.X, Alu.add)
        dve.tensor_tensor(OUTT[:, 0:1], SUMR[0:1, 0:1], SUMR[0:1, 3:4],
                          Alu.add)
        dve.tensor_tensor(OUTT[:, 0:1], OUTT[:, 0:1], s1[:], Alu.add)
        dve.tensor_tensor(OUTT[:, 1:2], SUMR[0:1, 1:2], SUMR[0:1, 4:5],
                          Alu.add)
        dve.tensor_reduce(s1[:], SUMR[0:1, 18:24], mybir.AxisListType.X,
                          Alu.add)
        dve.tensor_tensor(OUTT[:, 1:2], OUTT[:, 1:2], s1[:], Alu.subtract)
        dve.tensor_tensor(OUTT[:, 2:3], SUMR[0:1, 2:3], SUMR[0:1, 5:6],
                          Alu.add)
        dve.tensor_reduce(s1[:], np6[0:1, :], mybir.AxisListType.X, Alu.add)
        dve.tensor_copy(OUTT[:, 3:4], s1[:])
        dve.tensor_reduce(s1[:], K6[0:1, :], mybir.AxisListType.X, Alu.add)
        dve.tensor_copy(OUTT[:, 4:5], s1[:])
        nc.sync.dma_start(aps["out"], OUTT[:])


# =====================================================================
# host-side grid extraction + packing
# =====================================================================

_HOSTC = {}


def _extract_grid(anchors):
    """anchors: list of 3 [A,4] arrays. Returns dict or None if not grid."""
    out = {"X1": [], "X2": [], "Y1": [], "Y2": []}
    for s, (H, W, HW, L, co) in enumerate(SCALES):
        a4 = np.asarray(anchors[s], np.float32).reshape(H, W, 3, 4)
        x1 = a4[0, :, :, 0]          # [W,3]
        x2 = a4[0, :, :, 2]
        y1 = a4[:, 0, :, 1]          # [H,3]
        y2 = a4[:, 0, :, 3]
        if not (np.array_equal(a4[:, :, :, 0], np.broadcast_to(x1, (H, W, 3)))
                and np.array_equal(a4[:, :, :, 2],
                                   np.broadcast_to(x2, (H, W, 3)))
                and np.array_equal(a4[:, :, :, 1],
                                   np.broadcast_to(y1[:, None], (H, W, 3)))
                and np.array_equal(a4[:, :, :, 3],
                                   np.broadcast_to(y2[:, None], (H, W, 3)))):
            return None
        out["X1"].append(x1.T.copy())   # [3, W]
        out["X2"].append(x2.T.copy())
        out["Y1"].append(y1.T.copy())
        out["Y2"].append(y2.T.copy())
    return out


def _anchor_layout(vals, s):
    """[A] per-anchor values -> [128, 3L] tile block (col = a*L + g)."""
    H, W, HW, L, co = SCALES[s]
    return np.ascontiguousarray(
        vals.reshape(P, L, 3).transpose(0, 2, 1).reshape(P, 3 * L))


def _host_static(anchors):
    """Sample-independent packs: ancpk [128,4160] (SCL4|OFF4|Id), grid
    tables, area0 [3,3] (scale, a)."""
    key = "static"
    if key in _HOSTC:
        return _HOSTC[key]
    grid = _extract_grid(anchors)
    if grid is None:
        _HOSTC[key] = None
        return None
    scl4 = np.zeros((P, 2016), np.float32)
    off4 = np.zeros((P, 2016), np.float32)
    area0 = np.zeros((3, 3), np.float32)
    sx = np.float32(1.0 / QXY_S)
    sy = np.float32(1.0 / QXY_S)
    sw = np.float32(1.0 / QLN_S)
    sh = np.float32(1.0 / QLN_S)
    for s, (H, W, HW, L, co) in enumerate(SCALES):
        a4 = np.asarray(anchors[s], np.float32)
        aw = a4[:, 2] - a4[:, 0]
        ah = a4[:, 3] - a4[:, 1]
        acx = a4[:, 0] + np.float32(0.5) * aw
        acy = a4[:, 1] + np.float32(0.5) * ah
        area0[s] = (aw * ah)[0:3]
        rwa = (np.float32(1.0) / aw).astype(np.float32)
        rha = (np.float32(1.0) / ah).astype(np.float32)
        lnwa = np.log(aw).astype(np.float32)
        lnha = np.log(ah).astype(np.float32)
        scl = {0: sx * rwa, 1: sy * rha,
               2: np.full_like(rwa, sw), 3: np.full_like(rwa, sh)}
        off = {0: acx * rwa, 1: acy * rha,
               2: lnwa - np.float32(QLN_OFF), 3: lnha - np.float32(QLN_OFF)}
        for q in range(4):
            scl4[:, q * FCOL + co:q * FCOL + co + 3 * L] = _anchor_layout(
                scl[q].astype(np.float32), s)
            off4[:, q * FCOL + co:q * FCOL + co + 3 * L] = _anchor_layout(
                off[q].astype(np.float32), s)
    ancpk = np.concatenate([scl4, off4], axis=1)

    res = {"ancpk": np.ascontiguousarray(ancpk),
           "idm": np.eye(P, dtype=np.float32),
           "grid": grid, "area0": area0}
    _HOSTC[key] = res
    return res


def _host_percore(boxes_c, labels_c, static):
    """boxes_c [2,40,4], labels_c [2,40] -> tabpk [2,10,21,2528],
    smpk [128, 2*40*SPC] (packed content planes)."""
    area0 = static["area0"]
    grid = static["grid"]
    tabpk = np.zeros((SPC, 10, 21, 2528), np.float32)
    smpk = np.zeros((P, NBOX * SPC), np.float32)

    def tables(s, bx):
        """rw' [3,40,W], rh [3,40,H] for scale s (f32 stepwise)."""
        X1, X2 = grid["X1"][s], grid["X2"][s]
        Y1, Y2 = grid["Y1"][s], grid["Y2"][s]
        wb = bx[:, 2] - bx[:, 0]
        hb = bx[:, 3] - bx[:, 1]
        ab = wb * hb
        cs = (area0[s][:, None] + ab[None, :]).astype(np.float32) \
            + np.float32(1e-9)
        rcs = (np.float32(1.0) / cs).astype(np.float32)
        rw = np.minimum(X2[:, None, :], bx[None, :, 2:3]) \
            - np.maximum(X1[:, None, :], bx[None, :, 0:1])
        rw = np.maximum(rw, np.float32(0.0)) * rcs[:, :, None]
        rh = np.minimum(Y2[:, None, :], bx[None, :, 3:4]) \
            - np.maximum(Y1[:, None, :], bx[None, :, 1:2])
        rh = np.maximum(rh, np.float32(0.0))
        return rw.astype(np.float32), rh.astype(np.float32)

    pidx = np.arange(P)
    for b in range(SPC):
        bx = np.asarray(boxes_c[b], np.float32)
        wb = bx[:, 2] - bx[:, 0]
        hb = bx[:, 3] - bx[:, 1]
        rw0, rh0 = tables(0, bx)
        rw1, rh1 = tables(1, bx)
        rw2, rh2 = tables(2, bx)
        # scale1: lhsT[(a,par), p] = rh1[a,j,p//2]*(p%2==par); rhs
        # [(a,par),(a',g)] = delta(a,a')*rw1'[a,j,par*32+g]
        lh1 = np.zeros((NBOX, 6, 128), np.float32)
        rs1 = np.zeros((NBOX, 6, 96), np.float32)
        for a in range(3):
            for par in range(2):
                kk = a * 2 + par
                lh1[:, kk, :] = rh1[a][:, pidx // 2] * (pidx % 2 == par)
                rs1[:, kk, a * 32:(a + 1) * 32] = \
                    rw1[a][:, par * 32:(par + 1) * 32]
        lh2 = np.zeros((NBOX, 12, 128), np.float32)
        rs2 = np.zeros((NBOX, 12, 24), np.float32)
        for a in range(3):
            for qd in range(4):
                kk = a * 4 + qd
                lh2[:, kk, :] = rh2[a][:, pidx // 4] * (pidx % 4 == qd)
                rs2[:, kk, a * 8:(a + 1) * 8] = \
                    rw2[a][:, qd * 8:(qd + 1) * 8]
        for k in range(10):
            for slot in range(4):
                j = 4 * k + slot
                c0 = slot * FCOL
                for a in range(3):
                    tabpk[b, k, a, c0 + a * 128:c0 + (a + 1) * 128] = \
                        rw0[a, j]
                tabpk[b, k, 3:9, c0 + 384:c0 + 480] = rs1[j]
                tabpk[b, k, 9:21, c0 + 480:c0 + 504] = rs2[j]
                l0 = 2016 + slot * 128
                tabpk[b, k, 0:3, l0:l0 + 128] = rh0[:, j]
                tabpk[b, k, 3:9, l0:l0 + 128] = lh1[j]
                tabpk[b, k, 9:21, l0:l0 + 128] = lh2[j]
        # packed content: v = lab*2^22 + qx*2^15 + qy*2^8 + qw*2^4 + qh
        gcx = bx[:, 0] + np.float32(0.5) * wb
        gcy = bx[:, 1] + np.float32(0.5) * hb
        qx = np.clip(np.round(gcx * QXY_S), 0, 127).astype(np.float64)
        qy = np.clip(np.round(gcy * QXY_S), 0, 127).astype(np.float64)
        qw = np.clip(np.round((np.log(wb) - QLN_OFF) * QLN_S),
                     0, 15).astype(np.float64)
        qh = np.clip(np.round((np.log(hb) - QLN_OFF) * QLN_S),
                     0, 15).astype(np.float64)
        lab = np.asarray(labels_c[b], np.float64)
        p0 = (lab * float(2 ** 22) + qx * float(2 ** 15) + qy * 256.0
              + qw * 16.0 + qh).astype(np.float32)
        smpk[:, NBOX * b:NBOX * (b + 1)] = p0[None, :]
    return tabpk, smpk


# =====================================================================
# compile + run
# =====================================================================

_CACHE = {}


def _get_compiled_fast():
    if "fast" in _CACHE:
        return _CACHE["fast"]
    nc = bacc.Bacc("TRN2", target_bir_lowering=False, debug=False)
    aps = {
        "pred0": nc.dram_tensor("pred0", [SPC, 24, 128, 128], F32,
                                kind="ExternalInput").ap(),
        "pred1": nc.dram_tensor("pred1", [SPC, 24, 64, 64], F32,
                                kind="ExternalInput").ap(),
        "pred2": nc.dram_tensor("pred2", [SPC, 24, 32, 32], F32,
                                kind="ExternalInput").ap(),
        "ancpk": nc.dram_tensor("ancpk", [P, 4032], F32,
                                kind="ExternalInput").ap(),
        "idm": nc.dram_tensor("idm", [P, 128], F32R,
                              kind="ExternalInput").ap(),
        "tabpk": nc.dram_tensor("tabpk", [SPC, 10, 21, 2528], TAB_DT,
                                kind="ExternalInput").ap(),
        "smpk": nc.dram_tensor("smpk", [P, NBOX * SPC], F32,
                               kind="ExternalInput").ap(),
        "out": nc.dram_tensor("out", [1, 8], F32, kind="ExternalOutput").ap(),
    }
    with tile.TileContext(nc) as tc:
        _build_fast(tc, aps)
    nc.compile()
    _CACHE["fast"] = (nc, None)
    return _CACHE["fast"]


def _kernel_numpy(pred0, pred1, pred2, anchors0, anchors1, anchors2,
                  boxes, labels):
    """Self-contained numpy fallback (only for non-grid anchors)."""
    def softplus(x):
        return np.log1p(np.exp(-np.abs(x))) + np.maximum(x, 0.0)

    tot = np.zeros(5, np.float64)
    for pred, anc in ((pred0, anchors0), (pred1, anchors1),
                      (pred2, anchors2)):
        B, ch, H, W = pred.shape
        p = pred.transpose(0, 2, 3, 1).reshape(B, H * W * 3, 8)
        anc = np.asarray(anc, np.float64)
        aa = (anc[:, 2] - anc[:, 0]) * (anc[:, 3] - anc[:, 1])
        for b in range(B):
            bx = np.asarray(boxes[b], np.float64)
            ab = (bx[:, 2] - bx[:, 0]) * (bx[:, 3] - bx[:, 1])
            lt = np.maximum(anc[:, None, :2], bx[None, :, :2])
            rb = np.minimum(anc[:, None, 2:], bx[None, :, 2:])
            wh = np.clip(rb - lt, 0.0, None)
            inter = wh[..., 0] * wh[..., 1]
            iou = inter / (aa[:, None] + ab[None, :] - inter + 1e-9)
            best = iou.max(1)
            bidx = iou.argmax(1)
            pos = best >= 0.5
            neg = best < 0.3
            x = p[b, :, 4]
            oall = softplus(x) - x * pos
            npos = int(pos.sum())
            k = int(min(neg.sum(), 3 * max(npos, 1)))
            nl = np.where(neg, softplus(x), -1.0)
            order = np.argsort(-nl, kind="stable")
            sel = np.zeros(len(x), bool)
            sel[order[:k]] = True
            sel &= neg
            tot[0] += oall[pos | sel].sum()
            logit = p[b, :, 5:]
            m = logit.max(-1, keepdims=True)
            lse = np.log(np.exp(logit - m).sum(-1)) + m[:, 0]
            tgt = np.clip(labels[b][bidx] - 1, 0, 2)
            ce = lse - np.take_along_axis(logit, tgt[:, None], 1)[:, 0]
            tot[1] += ce[pos].sum()
            mb = bx[bidx]
            aw = anc[:, 2] - anc[:, 0]
            ah = anc[:, 3] - anc[:, 1]
            enc = np.stack([
                (0.5 * (mb[:, 0] + mb[:, 2]) - (anc[:, 0] + 0.5 * aw)) / aw,
                (0.5 * (mb[:, 1] + mb[:, 3]) - (anc[:, 1] + 0.5 * ah)) / ah,
                np.log((mb[:, 2] - mb[:, 0]) / aw),
                np.log((mb[:, 3] - mb[:, 1]) / ah)], -1)
            d = np.abs(p[b, :, :4] - enc)
            sl1 = np.where(d < 1.0, 0.5 * d * d, d - 0.5).sum(-1)
            tot[2] += sl1[pos].sum()
            tot[3] += npos
            tot[4] += int(sel.sum())
    norm = np.float32(max(tot[3], 1.0))
    lo = np.float32(tot[0] / norm)
    lc = np.float32(tot[1] / norm)
    ll = np.float32(tot[2] / norm)
    return (lo, lc, ll, np.float32(lo + lc + 2.0 * ll),
            np.float32(tot[3]), np.float32(tot[4]))


def kernel(pred0, pred1, pred2, anchors0, anchors1, anchors2, boxes, labels,
           _want_results=False, _trace=False):
    static = _host_static([anchors0, anchors1, anchors2])
    if static is None:   # pragma: no cover
        out = _kernel_numpy(pred0, pred1, pred2, anchors0, anchors1,
                            anchors2, boxes, labels)
        out = tuple(np.asarray(v, np.float32) for v in out)
        return (out, None) if _want_results else out
    nc, _ = _get_compiled_fast()
    in_maps = []
    for c in range(NCORES):
        sl = slice(c * SPC, (c + 1) * SPC)
        tabpk, smpk = _host_percore(boxes[sl], labels[sl], static)
        tabpk = tabpk.astype(ml_dtypes.bfloat16)
        in_maps.append({
            "pred0": np.ascontiguousarray(pred0[sl], np.float32),
            "pred1": np.ascontiguousarray(pred1[sl], np.float32),
            "pred2": np.ascontiguousarray(pred2[sl], np.float32),
            "ancpk": static["ancpk"],
            "idm": static["idm"],
            "tabpk": np.ascontiguousarray(tabpk),
            "smpk": np.ascontiguousarray(smpk),
        })
    res = bass_utils.run_bass_kernel_spmd(
        nc, in_maps, core_ids=list(range(NCORES)), trace=_trace)
    parts = np.stack([res.results[c]["out"][0] for c in range(NCORES)])
    tot = parts.sum(axis=0, dtype=np.float64).astype(np.float32)
    tot_obj, tot_cls, tot_loc, tot_pos, tot_neg = tot[:5]
    norm = np.float32(max(tot_pos, np.float32(1.0)))
    lo = np.float32(tot_obj / norm)
    lc = np.float32(tot_cls / norm)
    ll = np.float32(tot_loc / norm)
    ltot = np.float32(lo + lc + np.float32(2.0) * ll)
    out = (lo, lc, ll, ltot, np.float32(tot_pos), np.float32(tot_neg))
    out = tuple(np.asarray(v, np.float32) for v in out)
    if _want_results:
        return out, res
    return out


# revision 68
# speedup vs baseline: 1.1613x; 1.0295x over previous
"""Trainium2 Bass kernel for the 3-scale anchor DetectionLoss (fast path).

Sharding: data-parallel over batch (16 samples -> 8 cores x 2 samples).
Each core computes the six partial accumulators for its 2 samples; the
host sums the per-core partials and applies the global normalizer.

Fast-path algorithm (per core), v2 (engine-rebalanced two-pass):
- Score proxy: for anchor A and box B, x = inter/(areaA+areaB+1e-9) is a
  strictly monotone transform of IOU per pair, so pos/neg thresholds and
  the per-anchor argmax all come from x. All 3 scales' x-scores come from
  one K=21 bf16 block-diagonal matmul per box on the PE into PSUM
  (host-precomputed rw'/c tables streamed per 4-box chunk).
- Pass 1: PE fills 4-box score chunks (double-buffered PSUM); DVE only
  max-reduces them into BESTX (the per-anchor best score).
- Pass 2: PE recomputes the identical scores and accumulates -BESTX on
  top via an fp32r identity matmul; ACT converts the residual into bf16
  winner masks (Relu(y*K + 2)); DVE only does the matched-content gather:
  ONE copy_predicated of 2 packed fp32 planes per box (bcx|bcy and
  lnwb|lnhb|label, 12/12 and 11/11/2-bit fields).
- Losses: packed content decoded with mod/subtract; SmoothL1 via the
  identity 0.5*m^2 - m + |d| (m=min(|d|,1), Square/Abs on ACT); pos/neg
  masks computed per-scale with fused count accum_out; softplus on ACT
  chain; hard-negative mining via threshold bisection (unchanged).
- No GPSIMD (it shares SBUF ports with DVE and poisons its throughput).

Generic fallback: if the anchors are not a consistent grid, fall back to
a numpy implementation.
"""

import numpy as np
import ml_dtypes
from contextlib import ExitStack

import concourse.bass as bass
import concourse.tile as tile
from concourse import bacc, mybir
from concourse import bass_utils
from concourse import bass_isa
from concourse.dve_spec import (Spec, Src0, Src1, C0, C2, Zero, One,
                                sq, maxx, minn, select, eq, lower)
from concourse.dve_spec import AluOp as DAlu
from concourse.dve_ops import DveOp, OPS, DveOpSpec, get_dve_sub_opcode
from concourse.dve_ops import has_src1 as _has_src1


def _register_dve_op(name, spec, subdim=False):
    """Register a custom DVE op at import time, self-pinning its uop sha."""
    import concourse.dve_ops as _dom
    for ex in OPS:
        if ex.name == name:
            return ex
    op = DveOp(name, spec, subdim=subdim, uops_sha={})
    OPS.append(op)
    row = _dom._CUSTOM_DVE_ROW_BASE + len(OPS) - 1
    assert row < 0x20, "custom DVE opcode rows exhausted"
    _dom._SUB_OPCODE_FOR_NAME[name] = row
    _dom.CUSTOM_DVE_SPECS[name] = spec
    for ver in ("v3", "v4"):
        tmp = DveOpSpec(name=name, opcode=get_dve_sub_opcode(name),
                        uops=lower(spec, ver=ver), rd1_en=_has_src1(spec))
        op.uops_sha[ver] = tmp.sha(ver)
    return op


def _ref_sl1f(in0, in1, s0, s1, imm2):
    d = in0.astype(np.float32) + in1
    ad = np.abs(d)
    m = np.minimum(ad, np.float32(1.0))
    return ((m - 1.0) ** 2 * imm2 + ad).astype(np.float32)


def _ref_selsum(in0, in1, s0, s1, imm2):
    b = np.where(in1 == s0, in0.astype(np.float32), 0.0).astype(np.float32)
    return b, b.reshape(b.shape[0], -1).sum(axis=-1, keepdims=True)


def _ref_neglf(in0, in1, s0, s1, imm2):
    return ((in0.astype(np.float32) + 1.0) * in1 - 1.0).astype(np.float32)


_D = Src0 + Src1
_AD = maxx(_D, Zero - _D)
ANT_SL1F = _register_dve_op("ANT_SL1F", Spec(
    body=sq(minn(_AD, One) - One) * C2 + _AD,
    reference=_ref_sl1f))
ANT_SELSUM = _register_dve_op("ANT_SELSUM", Spec(
    body=select(eq(Src1, C0), Src0, Zero),
    accum=DAlu.ADD,
    reference=_ref_selsum))
ANT_NEGLF = _register_dve_op("ANT_NEGLF", Spec(
    body=(Src0 + One) * Src1 - One,
    reference=_ref_neglf))

F32 = mybir.dt.float32
F32R = mybir.dt.float32r
I32 = mybir.dt.int32
U8 = mybir.dt.uint8
BF16 = mybir.dt.bfloat16
TAB_DT = BF16
Alu = mybir.AluOpType
Act = mybir.ActivationFunctionType
Red = bass_isa.ReduceOp

NCORES = 8
SPC = 2          # samples per core
NBOX = 40
P = 128
FCOL = 504
NITER = 5        # bisection iterations for top-k threshold
MINE_LO = 0.0    # negl = softplus(x)+1 in [1,~6.5] for neg anchors, else -1
MINE_HI = 10.0
MASK_K = 1.0e6   # winner-mask scale: band = 128/MASK_K in score units

# (H, W, HW, L, col_off) ; L = locations per partition
SCALES = [
    (128, 128, 16384, 128, 0),
    (64, 64, 4096, 32, 384),
    (32, 32, 1024, 8, 480),
]
SCOLS = ((0, 384), (384, 480), (480, 504))
THR_POS = float(np.float32(1.0 / 3.0))
THR_NEG = float(np.float32(3.0 / 13.0))

# content quantization, ONE packed plane per box:
#   v = lab*2^22 + qx*2^15 + qy*2^8 + qw*2^4 + qh   (2|7|7|4|4 bits)
# lab is extracted exactly with two is_ge ops; the coordinate fields via
# floor(t) = round_to_int(t - (0.5 - 2^-(bl+1))) which stays exactly
# representable for each level's bit split.
QXY_S = 1.0 / 8.0    # px -> qx step 8
QLN_OFF = 2.9
QLN_S = 7.0          # ln -> q step 1/7
LAB_B = float(2.0 ** 22)


def _floor_c(bl):
    return float(np.float32(0.5 - 2.0 ** -(bl + 1)))


# =====================================================================
# fast device body
# =====================================================================

def _build_fast(tc, aps):
    nc = tc.nc
    dve = nc.vector
    act = nc.scalar
    pe = nc.tensor

    pred_aps = [aps["pred0"], aps["pred1"], aps["pred2"]]

    with ExitStack() as ctx:
        pstat = ctx.enter_context(tc.tile_pool(name="stat", bufs=1))
        pwork = ctx.enter_context(tc.tile_pool(name="work", bufs=1))
        pscr = ctx.enter_context(tc.tile_pool(name="scr", bufs=1))
        pbit = ctx.enter_context(tc.tile_pool(name="bit", bufs=3))
        pbt = ctx.enter_context(tc.tile_pool(name="bt", bufs=3))

        # ---------------- static tiles (DMAs deferred: the score-table
        # stream must hit the DMA queue first so pass 1 starts early) ----
        ANCPK = pstat.tile([P, 4032], F32, tag="ancpk", name="ancpk")
        SCL4 = ANCPK[:, 0:2016]          # content dequant scales x|y|w|h
        OFF4 = ANCPK[:, 2016:4032]       # acx*rwa | acy*rha | lnwa-c | lnha-c
        IDM = pstat.tile([P, 128], F32R, tag="idm", name="idm")
        SMPK = pstat.tile([P, NBOX * SPC], F32, tag="smpk", name="smpk")

        def static_dma():
            # separate trigger engine -> separate DMA queue, so these bulk
            # loads don't head-block the per-chunk score-table stream
            nc.gpsimd.dma_start(IDM[:], aps["idm"])
            nc.gpsimd.dma_start(SMPK[:], aps["smpk"])
            nc.gpsimd.dma_start(ANCPK[:], aps["ancpk"])

        PREDB = [pstat.tile([P, 4032], F32, tag=f"pred{b}", name=f"pred{b}")
                 for b in range(SPC)]

        def pred_dma(b):
            for s, (H, W, HW, L, co) in enumerate(SCALES):
                for a in range(3):
                    s_v = pred_aps[s][b, a * 8:(a + 1) * 8].rearrange(
                        "f h w -> f (h w)").rearrange(
                        "f (p g) -> p f g", p=P)
                    d_v = PREDB[b][:].rearrange(
                        "p (f c) -> p f c", f=8)[:, :, co + a * L:
                                                 co + (a + 1) * L]
                    nc.gpsimd.dma_start(d_v, s_v)

        ONES128 = pstat.tile([P, 1], F32, tag="o128", name="o128")
        nc.gpsimd.memset(ONES128[:], 1.0)
        ALL1 = pstat.tile([P, 128], F32, tag="all1", name="all1")
        nc.gpsimd.memset(ALL1[:], 1.0)
        B128 = pstat.tile([P, 1], F32, tag="b128", name="b128")
        nc.gpsimd.memset(B128[:], 128.0)
        BN7 = pstat.tile([P, 1], F32, tag="bn7", name="bn7")
        nc.gpsimd.memset(BN7[:], -0.7071067811865476)
        ONES1 = pstat.tile([1, 128], F32, tag="o1", name="o1")
        nc.gpsimd.memset(ONES1[:], 1.0)

        # ---------------- persistent working tiles ----------------
        BESTX = pwork.tile([P, 1008], F32, tag="bestx", name="bestx")
        nc.gpsimd.memset(BESTX[:], 0.0)
        BESTYM = pwork.tile([P, 1008], F32R, tag="bestym", name="bestym")
        POSA = pwork.tile([P, 1008], F32, tag="posa", name="posa")
        NEGA = pwork.tile([P, 1008], F32, tag="nega", name="nega")
        NEGL = pwork.tile([P, 1008], F32, tag="negl", name="negl")
        # matched content: 1 packed plane (shared by samples; DVE program
        # order serializes sample0 losses before sample1 gather)
        MQ2 = pwork.tile([P, FCOL], F32, tag="mq2", name="mq2")
        MLAB = pwork.tile([P, FCOL], F32, tag="mlab", name="mlab")
        QI = pwork.tile([P, FCOL], I32, tag="qi", name="qi")
        LSE = pwork.tile([P, 1008], F32, tag="lse", name="lse")
        # partial accumulators: cols 0-5 obj/cls/loc per sample,
        # 6-11 npos(b,s), 12-17 nneg(b,s), 18-23 CE picked-logit sums (b,c)
        PARTALL = pwork.tile([P, 24], F32, tag="partall", name="partall")

        BIG = [pscr.tile([P, 4032], F32, tag=f"big{i}", name=f"big{i}")
               for i in range(3)]

        ppsum = None   # bound inside the psA pool scope below

        # ---------------- score chunk matmuls (4 boxes) ----------------
        def mm_chunk(PS, b, k, stop):
            twh = pbt.tile([21, 2528], TAB_DT, tag="twh", name="twh")
            nc.sync.dma_start(twh[:], aps["tabpk"][b, k])
            for slot in range(4):
                pe.matmul(PS[:, slot * 512:slot * 512 + FCOL],
                          twh[0:21, 2016 + slot * 128:
                              2016 + (slot + 1) * 128],
                          twh[0:21, slot * FCOL:(slot + 1) * FCOL],
                          start=True, stop=stop)

        # ---------------- pass 1: best score ----------------
        def pass1(b, mid_cb=None):
            bx = BESTX[:, b * FCOL:(b + 1) * FCOL]
            red = BIG[1][:, 0:FCOL]
            for k in range(10):
                if k == 1 and mid_cb is not None:
                    mid_cb()
                PS = ppsum.tile([P, 2048], F32, tag="ps", name="ps")
                mm_chunk(PS, b, k, stop=True)
                v = PS[:].rearrange("p (s c) -> p c s", s=4)[:, 0:FCOL, :]
                dve.tensor_reduce(red, v, mybir.AxisListType.X, Alu.max)
                dve.tensor_tensor(bx, bx, red, Alu.max)

        # ---------------- masks + per-scale counts + BESTYM ----------
        def masks(b):
            for s, (c0, c1) in enumerate(SCOLS):
                dve.tensor_scalar(
                    POSA[:, b * FCOL + c0:b * FCOL + c1],
                    BESTX[:, b * FCOL + c0:b * FCOL + c1],
                    THR_POS, 0.0, Alu.is_ge, Alu.add,
                    accum_out=PARTALL[:, 6 + 3 * b + s:7 + 3 * b + s])
                dve.tensor_scalar(
                    NEGA[:, b * FCOL + c0:b * FCOL + c1],
                    BESTX[:, b * FCOL + c0:b * FCOL + c1],
                    THR_NEG, 0.0, Alu.is_lt, Alu.add,
                    accum_out=PARTALL[:, 12 + 3 * b + s:13 + 3 * b + s])
            dve.tensor_scalar(BESTYM[:, b * FCOL:(b + 1) * FCOL],
                              BESTX[:, b * FCOL:(b + 1) * FCOL],
                              -1.0, None, Alu.mult)

        # ---------------- pass 2: winner masks + content gather -------
        def pass2(b, stage_cb=None):
            bymr = BESTYM[:, b * FCOL:(b + 1) * FCOL]
            idmr = IDM[:]
            cv = SMPK[:, NBOX * b:NBOX * (b + 1)]
            for k in range(10):
                PS = ppsum.tile([P, 2048], F32, tag="ps", name="ps")
                mm_chunk(PS, b, k, stop=False)
                for slot in range(4):
                    pe.matmul(PS[:, slot * 512:slot * 512 + FCOL],
                              idmr, bymr, start=False, stop=True)
                bt = pbit.tile([P, 4 * FCOL], U8, tag="bit", name="bit")
                btv = bt[:].rearrange("p (s c) -> p s c", s=4)
                psv = PS[:].rearrange("p (s c) -> p s c", s=4)[:, :, 0:FCOL]
                act.activation(btv, psv, Act.Relu, bias=B128[:],
                               scale=MASK_K)
                # one gather for the whole chunk: the out AP revisits the
                # same [P,504] region per slot (slot-major stream order),
                # so later boxes overwrite earlier ones like the per-box
                # sequence did
                dve.copy_predicated(
                    MQ2[:].unsqueeze(1).broadcast_to([P, 4, FCOL]),
                    btv,
                    cv[:, 4 * k:4 * k + 4].unsqueeze(2).broadcast_to(
                        [P, 4, FCOL]))
                if stage_cb is not None:
                    stage_cb(k)

        # ---------------- early per-sample pieces (only need PREDB +
        # masks): obj BCE accumulation, NEGL for mining, and the CE
        # log-sum-exp — keeps ACT's exp/ln chain ahead of the mask stream.
        def objneg(b):
            posb = POSA[:, b * FCOL:(b + 1) * FCOL]
            negb = NEGA[:, b * FCOL:(b + 1) * FCOL]
            X = PREDB[b][:, 4 * FCOL:5 * FCOL]
            ax = BIG[1][:, 0:FCOL]
            ex = BIG[1][:, 504:1008]
            sp = BIG[1][:, 1008:1512]
            cacc = BIG[1][:, 1512:2016]
            act.activation(ax, X, Act.Abs)
            act.activation(ex, ax, Act.Exp, scale=-1.0)
            act.activation(ax, ex, Act.Ln, bias=1.0)
            dve.scalar_tensor_tensor(sp, X, 0.0, ax,
                                     Alu.max, Alu.add)
            dve.tensor_tensor(ex, sp, X, Alu.subtract)
            dve.scalar_tensor_tensor(cacc, ex, 0.0, posb,
                                     Alu.add, Alu.mult,
                                     accum_out=PARTALL[:, 3 * b:3 * b + 1])
            nb = NEGL[:, b * FCOL:(b + 1) * FCOL]
            dve._custom_dve(ANT_NEGLF, out=nb, in0=sp, in1=negb)
            # CE log-sum-exp (kept per sample in its own slot)
            CL0 = PREDB[b][:, 5 * FCOL:6 * FCOL]
            CL1 = PREDB[b][:, 6 * FCOL:7 * FCOL]
            CL2 = PREDB[b][:, 7 * FCOL:8 * FCOL]
            lse = LSE[:, b * FCOL:(b + 1) * FCOL]
            e1 = BIG[1][:, 0:FCOL]
            act.activation(lse, CL0, Act.Exp)
            act.activation(e1, CL1, Act.Exp)
            dve.tensor_tensor(lse, lse, e1, Alu.add)
            act.activation(e1, CL2, Act.Exp)
            dve.tensor_tensor(lse, lse, e1, Alu.add)
            act.activation(lse, lse, Act.Ln)

        # ---------------- per-sample losses ----------------
        def losses(b):
            posb = POSA[:, b * FCOL:(b + 1) * FCOL]
            g1 = MQ2[:, 0:FCOL]

            # ----- decode packed content -----
            # lab exactly via two is_ge thresholds; coordinate fields via
            # fused round-to-int floor extraction per level.
            CONT = BIG[0][:, 0:2016]
            CONTI = CONT.bitcast(I32)
            qxi = CONTI[:, 0:504]
            qyi = CONTI[:, 504:1008]
            qwi = CONTI[:, 1008:1512]
            qhi = CONTI[:, 1512:2016]
            t2 = BIG[1][:, 0:FCOL]
            rem = QI[:]
            dve.tensor_scalar(MLAB[:], g1, 2.0 * LAB_B, None, Alu.is_ge)
            dve.tensor_scalar(t2, g1, 3.0 * LAB_B, None, Alu.is_ge)
            dve.scalar_tensor_tensor(MLAB[:], MLAB[:], 1.0, t2,
                                     Alu.add, Alu.add)
            dve.scalar_tensor_tensor(rem, MLAB[:], -LAB_B, g1,
                                     Alu.mult, Alu.add)
            dve.tensor_scalar(qxi, rem, 2.0 ** -15, -_floor_c(15),
                              Alu.mult, Alu.add)
            dve.scalar_tensor_tensor(rem, qxi, -float(2 ** 15), rem,
                                     Alu.mult, Alu.add)
            dve.tensor_scalar(qyi, rem, 2.0 ** -8, -_floor_c(8),
                              Alu.mult, Alu.add)
            dve.scalar_tensor_tensor(rem, qyi, -256.0, rem,
                                     Alu.mult, Alu.add)
            dve.tensor_scalar(qwi, rem, 2.0 ** -4, -_floor_c(4),
                              Alu.mult, Alu.add)
            dve.scalar_tensor_tensor(qhi, qwi, -16.0, rem,
                                     Alu.mult, Alu.add)

            # ----- loc (SmoothL1) -----
            # sl1 = 0.5*min(|d|,1)^2 - min(|d|,1) + |d|
            #     = 0.5*(m-1)^2 + |d| - 0.5, with the -0.5 folded into the
            #       masked accumulation below (scalar -2.0 over 4 planes)
            #       and the rest fused into one custom DVE op (ANT_SL1F).
            T1 = BIG[2][:, 0:2016]
            W = BIG[2][:, 2016:4032]
            dve.tensor_tensor(T1, CONTI, SCL4, Alu.mult)
            dve.tensor_tensor(W, PREDB[b][:, 0:2016], T1, Alu.subtract)
            SL = CONT  # reuse
            dve._custom_dve(ANT_SL1F, out=SL, in0=W, in1=OFF4, imm2=0.5)
            s2 = BIG[1][:, 2520:3528]
            dve.tensor_tensor(s2, SL[:, 0:1008], SL[:, 1008:2016], Alu.add)
            sl = BIG[1][:, 3528:4032]
            dve.tensor_tensor(sl, s2[:, 0:504], s2[:, 504:1008], Alu.add)
            cacc = BIG[1][:, 0:FCOL]
            dve.scalar_tensor_tensor(cacc, sl, -2.0, posb,
                                     Alu.add, Alu.mult,
                                     accum_out=PARTALL[:, 3 * b + 2:
                                                       3 * b + 3])

            # ----- CE: sum_pos lse accumulated positively; the picked
            # class logit accumulated per class into cols 18-23 (subtracted
            # in the final combine) via the custom select-eq-sum op. -----
            CL0 = PREDB[b][:, 5 * FCOL:6 * FCOL]
            CL1 = PREDB[b][:, 6 * FCOL:7 * FCOL]
            CL2 = PREDB[b][:, 7 * FCOL:8 * FCOL]
            mlp = BIG[0][:, 2016:2520]
            selscr = BIG[0][:, 2520:3024]
            lse = LSE[:, b * FCOL:(b + 1) * FCOL]
            dve.scalar_tensor_tensor(cacc, lse, 0.0, posb,
                                     Alu.add, Alu.mult,
                                     accum_out=PARTALL[:, 3 * b + 1:
                                                       3 * b + 2])
            dve.tensor_tensor(mlp, MLAB[:], posb, Alu.mult)
            for c, CLp in enumerate((CL0, CL1, CL2)):
                dve._custom_dve(
                    ANT_SELSUM, out=selscr, in0=CLp, in1=mlp,
                    s0=float(c + 1),
                    accum_out=PARTALL[:, 18 + 3 * b + c:19 + 3 * b + c])

        # ================= hard-negative mining =================
        # Bisection with replicated [P,6] state, interleaved through the
        # pass-2 emission. Per-segment counts come from ACT via the Sign
        # trick: count(>thr) = (sum sign(negl-thr) + Ntot)/2 (non-neg
        # anchors hold negl=-1 and contribute -1 each, absorbed by Ntot).
        # Cross-partition sums via GPSIMD partition_all_reduce (no PSUM).
        gp = nc.gpsimd
        t6 = lambda n: pwork.tile([P, 6], F32, tag=n, name=n)
        K6 = t6("k6")
        K2W = t6("k2w")
        WTOT = t6("wtot")
        LO = t6("lo6")
        HI = t6("hi6")
        MID = t6("mid6")
        NMID = t6("nmid6")
        GTK = t6("gtk6")
        DD = t6("dd6")
        CNTA = t6("cnta")
        CNTR = t6("cntr")
        SG = t6("sg6")
        KK = t6("kk6")
        HSEL = t6("hsel")
        HSEL2 = t6("hsel2")
        NP12R = pwork.tile([P, 12], F32, tag="np12r", name="np12r")
        CG12 = pwork.tile([P, 12], F32, tag="cg12", name="cg12")
        CGR = pwork.tile([P, 12], F32, tag="cgr", name="cgr")
        MSCR = pwork.tile([P, 384], F32, tag="mscr", name="mscr")
        MSCR2 = pwork.tile([P, 96], F32, tag="mscr2", name="mscr2")
        SEGS = [(b, c0, c1) for b in range(SPC) for (c0, c1) in SCOLS]
        # big (384-col) segments counted on ACT via the Sign trick; the
        # small ones on DVE with direct is_gt counts. WSEL/HSEL convert the
        # mixed representations to counts: cnt = (raw + WSEL) * HSEL.
        ACT_SEG = [i for i, (b, c0, c1) in enumerate(SEGS) if c1 - c0 >= 384]

        def mine_prep():
            for i, (b, c0, c1) in enumerate(SEGS):
                w = float(P * (c1 - c0)) if i in ACT_SEG else 0.0
                nc.gpsimd.memset(WTOT[:, i:i + 1], w)
                nc.gpsimd.memset(HSEL[:, i:i + 1],
                                 0.5 if i in ACT_SEG else 1.0)
                nc.gpsimd.memset(HSEL2[:, i:i + 1],
                                 1.0 if i in ACT_SEG else 0.0)
            gp.partition_all_reduce(NP12R[:], PARTALL[:, 6:18], P, Red.add)
            dve.tensor_scalar(K6[:], NP12R[:, 0:6], 1.0, 3.0,
                              Alu.max, Alu.mult)
            dve.tensor_tensor(K6[:], K6[:], NP12R[:, 6:12], Alu.min)
            # ACT segments compare sign-sums against 2K - Ntot; DVE ones
            # compare counts against K
            dve.tensor_scalar(K2W[:], K6[:], 2.0, None, Alu.mult)
            dve.tensor_tensor(K2W[:], K2W[:], WTOT[:], Alu.subtract)
            for i in range(6):
                if i not in ACT_SEG:
                    dve.tensor_copy(K2W[:, i:i + 1], K6[:, i:i + 1])
            dve.memset(LO[:], MINE_LO)
            dve.memset(HI[:], MINE_HI)
            mine_stageA(0)

        def mine_stageA(it):
            dve.tensor_tensor(MID[:], LO[:], HI[:], Alu.add)
            dve.tensor_scalar(MID[:], MID[:], 0.5, None, Alu.mult)
            dve.tensor_scalar(NMID[:], MID[:], -1.0, None, Alu.mult)
            for i, (b, c0, c1) in enumerate(SEGS):
                sl_ = NEGL[:, b * FCOL + c0:b * FCOL + c1]
                if i in ACT_SEG:
                    act.activation(MSCR[:, 0:c1 - c0], sl_,
                                   Act.Sign, bias=NMID[:, i:i + 1],
                                   accum_out=CNTA[:, i:i + 1])
                else:
                    dve.tensor_scalar(MSCR2[:, 0:c1 - c0], sl_,
                                      MID[:, i:i + 1], 0.0,
                                      Alu.is_gt, Alu.add,
                                      accum_out=CNTA[:, i:i + 1])
            gp.partition_all_reduce(CNTR[:], CNTA[:], P, Red.add)

        def mine_stageB(it):
            dve.tensor_tensor(GTK[:], CNTR[:], K2W[:], Alu.is_gt)
            dve.tensor_tensor(DD[:], MID[:], LO[:], Alu.subtract)
            dve.tensor_tensor(DD[:], GTK[:], DD[:], Alu.mult)
            dve.tensor_tensor(LO[:], LO[:], DD[:], Alu.add)
            dve.tensor_tensor(DD[:], HI[:], MID[:], Alu.subtract)
            dve.tensor_tensor(DD[:], GTK[:], DD[:], Alu.mult)
            dve.tensor_tensor(HI[:], MID[:], DD[:], Alu.add)

        def mine_final_acts():
            dve.tensor_scalar(NMID[:], HI[:], -1.0, None, Alu.mult)
            for i, (b, c0, c1) in enumerate(SEGS):
                sl_ = NEGL[:, b * FCOL + c0:b * FCOL + c1]
                if i in ACT_SEG:
                    act.activation(MSCR[:, 0:c1 - c0], sl_, Act.Sign,
                                   bias=NMID[:, i:i + 1],
                                   accum_out=CG12[:, i:i + 1])
                    act.activation(MSCR[:, 0:c1 - c0], sl_, Act.Relu,
                                   bias=NMID[:, i:i + 1],
                                   accum_out=CG12[:, 6 + i:7 + i])
                else:
                    dve.tensor_scalar(MSCR2[:, 0:c1 - c0], sl_,
                                      HI[:, i:i + 1], 0.0,
                                      Alu.is_gt, Alu.add,
                                      accum_out=CG12[:, i:i + 1])
                    dve.scalar_tensor_tensor(MSCR2[:, 0:c1 - c0], sl_,
                                             HI[:, i:i + 1], sl_,
                                             Alu.is_gt, Alu.mult,
                                             accum_out=CG12[:, 6 + i:7 + i])
            gp.partition_all_reduce(CGR[:], CG12[:], P, Red.add)

        def mine_final_kk():
            # cnt(>HI): ACT segs (sgn_sum + Ntot)/2, DVE segs raw count
            dve.tensor_tensor(CNTR[:], CGR[:, 0:6], WTOT[:], Alu.add)
            dve.tensor_tensor(CNTR[:], CNTR[:], HSEL[:], Alu.mult)
            # S(>HI): ACT segs relu_sum + cnt*HI, DVE segs direct sum
            dve.tensor_tensor(SG[:], CNTR[:], HI[:], Alu.mult)
            dve.tensor_tensor(SG[:], SG[:], HSEL2[:], Alu.mult)
            dve.tensor_tensor(SG[:], SG[:], CGR[:, 6:12], Alu.add)
            # stragglers lie in (LO, HI]; estimate by the interval midpoint
            dve.tensor_tensor(MID[:], LO[:], HI[:], Alu.add)
            dve.tensor_scalar(MID[:], MID[:], 0.5, None, Alu.mult)
            dve.tensor_tensor(KK[:], K6[:], CNTR[:], Alu.subtract)
            dve.tensor_tensor(KK[:], KK[:], MID[:], Alu.mult)
            dve.tensor_tensor(KK[:], KK[:], SG[:], Alu.add)

        def mine_stage0(k):
            # A(0) ran at prep; B(i)/A(i+1) at odd chunks, finals at k=9
            if k % 2 == 1:
                it = (k - 1) // 2
                if it < NITER:
                    mine_stageB(it)
                    if it + 1 < NITER:
                        mine_stageA(it + 1)
                    elif it + 1 == NITER:
                        mine_final_acts()

        def mine_stage1(k):
            if k == 1:
                mine_final_kk()

        # ================= emit the pipeline =================
        with tc.psum_pool(name="psA", bufs=2) as psA:
            ppsum = psA
            static_dma()
            pass1(0, mid_cb=lambda: pred_dma(0))
            masks(0)
            objneg(0)
            pass1(1, mid_cb=lambda: pred_dma(1))
            masks(1)
            objneg(1)
            mine_prep()
            pass2(0, stage_cb=mine_stage0)
            losses(0)
            pass2(1, stage_cb=mine_stage1)
            losses(1)

        # ================= final cross-partition sums =================
        ppsB = ctx.enter_context(tc.psum_pool(name="psB", bufs=1))
        SUMP = ppsB.tile([P, 24], F32, tag="sump", name="sump")
        pe.matmul(SUMP[:], ALL1[:], PARTALL[:])
        SUMR = pwork.tile([P, 24], F32, tag="sumr", name="sumr")
        dve.tensor_copy(SUMR[:], SUMP[:])
        np6 = SUMR[:, 6:12]

        # ---------------- final combine + store ----------------
        OUTT = pwork.tile([1, 8], F32, tag="outt", name="outt")
        s1 = pwork.tile([1, 1], F32, tag="s1", name="s1")
        # obj = objp0 + objp1 + sum(KK)
        dve.tensor_reduce(s1[:], KK[0:1, :], mybir.AxisListType.X, Alu.add)
        dve.tensor_tensor(OUTT[:, 0:1], SUMR[0:1, 0:1], SUMR[0:1, 3:4],
                          Alu.add)
        dve.tensor_tensor(OUTT[:, 0:1], OUTT[:, 0:1], s1[:], Alu.add)
        dve.tensor_tensor(OUTT[:, 1:2], SUMR[0:1, 1:2], SUMR[0:1, 4:5],
                          Alu.add)
        dve.tensor_reduce(s1[:], SUMR[0:1, 18:24], mybir.AxisListType.X,
                          Alu.add)
        dve.tensor_tensor(OUTT[:, 1:2], OUTT[:, 1:2], s1[:], Alu.subtract)
        dve.tensor_tensor(OUTT[:, 2:3], SUMR[0:1, 2:3], SUMR[0:1, 5:6],
                          Alu.add)
        dve.tensor_reduce(s1[:], np6[0:1, :], mybir.AxisListType.X, Alu.add)
        dve.tensor_copy(OUTT[:, 3:4], s1[:])
        dve.tensor_reduce(s1[:], K6[0:1, :], mybir.AxisListType.X, Alu.add)
        dve.tensor_copy(OUTT[:, 4:5], s1[:])
        nc.sync.dma_start(aps["out"], OUTT[:])


# =====================================================================
# host-side grid extraction + packing
# =====================================================================

_HOSTC = {}


def _extract_grid(anchors):
    """anchors: list of 3 [A,4] arrays. Returns dict or None if not grid."""
    out = {"X1": [], "X2": [], "Y1": [], "Y2": []}
    for s, (H, W, HW, L, co) in enumerate(SCALES):
        a4 = np.asarray(anchors[s], np.float32).reshape(H, W, 3, 4)
        x1 = a4[0, :, :, 0]          # [W,3]
        x2 = a4[0, :, :, 2]
        y1 = a4[:, 0, :, 1]          # [H,3]
        y2 = a4[:, 0, :, 3]
        if not (np.array_equal(a4[:, :, :, 0], np.broadcast_to(x1, (H, W, 3)))
                and np.array_equal(a4[:, :, :, 2],
                                   np.broadcast_to(x2, (H, W, 3)))
                and np.array_equal(a4[:, :, :, 1],
                                   np.broadcast_to(y1[:, None], (H, W, 3)))
                and np.array_equal(a4[:, :, :, 3],
                                   np.broadcast_to(y2[:, None], (H, W, 3)))):
            return None
        out["X1"].append(x1.T.copy())   # [3, W]
        out["X2"].append(x2.T.copy())
        out["Y1"].append(y1.T.copy())
        out["Y2"].append(y2.T.copy())
    return out


def _anchor_layout(vals, s):
    """[A] per-anchor values -> [128, 3L] tile block (col = a*L + g)."""
    H, W, HW, L, co = SCALES[s]
    return np.ascontiguousarray(
        vals.reshape(P, L, 3).transpose(0, 2, 1).reshape(P, 3 * L))


def _host_static(anchors):
    """Sample-independent packs: ancpk [128,4160] (SCL4|OFF4|Id), grid
    tables, area0 [3,3] (scale, a)."""
    key = "static"
    if key in _HOSTC:
        return _HOSTC[key]
    grid = _extract_grid(anchors)
    if grid is None:
        _HOSTC[key] = None
        return None
    scl4 = np.zeros((P, 2016), np.float32)
    off4 = np.zeros((P, 2016), np.float32)
    area0 = np.zeros((3, 3), np.float32)
    sx = np.float32(1.0 / QXY_S)
    sy = np.float32(1.0 / QXY_S)
    sw = np.float32(1.0 / QLN_S)
    sh = np.float32(1.0 / QLN_S)
    for s, (H, W, HW, L, co) in enumerate(SCALES):
        a4 = np.asarray(anchors[s], np.float32)
        aw = a4[:, 2] - a4[:, 0]
        ah = a4[:, 3] - a4[:, 1]
        acx = a4[:, 0] + np.float32(0.5) * aw
        acy = a4[:, 1] + np.float32(0.5) * ah
        area0[s] = (aw * ah)[0:3]
        rwa = (np.float32(1.0) / aw).astype(np.float32)
        rha = (np.float32(1.0) / ah).astype(np.float32)
        lnwa = np.log(aw).astype(np.float32)
        lnha = np.log(ah).astype(np.float32)
        scl = {0: sx * rwa, 1: sy * rha,
               2: np.full_like(rwa, sw), 3: np.full_like(rwa, sh)}
        off = {0: acx * rwa, 1: acy * rha,
               2: lnwa - np.float32(QLN_OFF), 3: lnha - np.float32(QLN_OFF)}
        for q in range(4):
            scl4[:, q * FCOL + co:q * FCOL + co + 3 * L] = _anchor_layout(
                scl[q].astype(np.float32), s)
            off4[:, q * FCOL + co:q * FCOL + co + 3 * L] = _anchor_layout(
                off[q].astype(np.float32), s)
    ancpk = np.concatenate([scl4, off4], axis=1)

    res = {"ancpk": np.ascontiguousarray(ancpk),
           "idm": np.eye(P, dtype=np.float32),
           "grid": grid, "area0": area0}
    _HOSTC[key] = res
    return res


def _host_percore(boxes_c, labels_c, static):
    """boxes_c [2,40,4], labels_c [2,40] -> tabpk [2,10,21,2528],
    smpk [128, 2*40*SPC] (packed content planes)."""
    area0 = static["area0"]
    grid = static["grid"]
    tabpk = np.zeros((SPC, 10, 21, 2528), np.float32)
    smpk = np.zeros((P, NBOX * SPC), np.float32)

    def tables(s, bx):
        """rw' [3,40,W], rh [3,40,H] for scale s (f32 stepwise)."""
        X1, X2 = grid["X1"][s], grid["X2"][s]
        Y1, Y2 = grid["Y1"][s], grid["Y2"][s]
        wb = bx[:, 2] - bx[:, 0]
        hb = bx[:, 3] - bx[:, 1]
        ab = wb * hb
        cs = (area0[s][:, None] + ab[None, :]).astype(np.float32) \
            + np.float32(1e-9)
        rcs = (np.float32(1.0) / cs).astype(np.float32)
        rw = np.minimum(X2[:, None, :], bx[None, :, 2:3]) \
            - np.maximum(X1[:, None, :], bx[None, :, 0:1])
        rw = np.maximum(rw, np.float32(0.0)) * rcs[:, :, None]
        rh = np.minimum(Y2[:, None, :], bx[None, :, 3:4]) \
            - np.maximum(Y1[:, None, :], bx[None, :, 1:2])
        rh = np.maximum(rh, np.float32(0.0))
        return rw.astype(np.float32), rh.astype(np.float32)

    pidx = np.arange(P)
    for b in range(SPC):
        bx = np.asarray(boxes_c[b], np.float32)
        wb = bx[:, 2] - bx[:, 0]
        hb = bx[:, 3] - bx[:, 1]
        rw0, rh0 = tables(0, bx)
        rw1, rh1 = tables(1, bx)
        rw2, rh2 = tables(2, bx)
        # scale1: lhsT[(a,par), p] = rh1[a,j,p//2]*(p%2==par); rhs
        # [(a,par),(a',g)] = delta(a,a')*rw1'[a,j,par*32+g]
        lh1 = np.zeros((NBOX, 6, 128), np.float32)
        rs1 = np.zeros((NBOX, 6, 96), np.float32)
        for a in range(3):
            for par in range(2):
                kk = a * 2 + par
                lh1[:, kk, :] = rh1[a][:, pidx // 2] * (pidx % 2 == par)
                rs1[:, kk, a * 32:(a + 1) * 32] = \
                    rw1[a][:, par * 32:(par + 1) * 32]
        lh2 = np.zeros((NBOX, 12, 128), np.float32)
        rs2 = np.zeros((NBOX, 12, 24), np.float32)
        for a in range(3):
            for qd in range(4):
                kk = a * 4 + qd
                lh2[:, kk, :] = rh2[a][:, pidx // 4] * (pidx % 4 == qd)
                rs2[:, kk, a * 8:(a + 1) * 8] = \
                    rw2[a][:, qd * 8:(qd + 1) * 8]
        for k in range(10):
            for slot in range(4):
                j = 4 * k + slot
                c0 = slot * FCOL
                for a in range(3):
                    tabpk[b, k, a, c0 + a * 128:c0 + (a + 1) * 128] = \
                        rw0[a, j]
                tabpk[b, k, 3:9, c0 + 384:c0 + 480] = rs1[j]
                tabpk[b, k, 9:21, c0 + 480:c0 + 504] = rs2[j]
                l0 = 2016 + slot * 128
                tabpk[b, k, 0:3, l0:l0 + 128] = rh0[:, j]
                tabpk[b, k, 3:9, l0:l0 + 128] = lh1[j]
                tabpk[b, k, 9:21, l0:l0 + 128] = lh2[j]
        # packed content: v = lab*2^22 + qx*2^15 + qy*2^8 + qw*2^4 + qh
        gcx = bx[:, 0] + np.float32(0.5) * wb
        gcy = bx[:, 1] + np.float32(0.5) * hb
        qx = np.clip(np.round(gcx * QXY_S), 0, 127).astype(np.float64)
        qy = np.clip(np.round(gcy * QXY_S), 0, 127).astype(np.float64)
        qw = np.clip(np.round((np.log(wb) - QLN_OFF) * QLN_S),
                     0, 15).astype(np.float64)
        qh = np.clip(np.round((np.log(hb) - QLN_OFF) * QLN_S),
                     0, 15).astype(np.float64)
        lab = np.asarray(labels_c[b], np.float64)
        p0 = (lab * float(2 ** 22) + qx * float(2 ** 15) + qy * 256.0
              + qw * 16.0 + qh).astype(np.float32)
        smpk[:, NBOX * b:NBOX * (b + 1)] = p0[None, :]
    return tabpk, smpk


# =====================================================================
# compile + run
# =====================================================================

_CACHE = {}


def _get_compiled_fast():
    if "fast" in _CACHE:
        return _CACHE["fast"]
    nc = bacc.Bacc("TRN2", target_bir_lowering=False, debug=False)
    aps = {
        "pred0": nc.dram_tensor("pred0", [SPC, 24, 128, 128], F32,
                                kind="ExternalInput").ap(),
        "pred1": nc.dram_tensor("pred1", [SPC, 24, 64, 64], F32,
                                kind="ExternalInput").ap(),
        "pred2": nc.dram_tensor("pred2", [SPC, 24, 32, 32], F32,
                                kind="ExternalInput").ap(),
        "ancpk": nc.dram_tensor("ancpk", [P, 4032], F32,
                                kind="ExternalInput").ap(),
        "idm": nc.dram_tensor("idm", [P, 128], F32R,
                              kind="ExternalInput").ap(),
        "tabpk": nc.dram_tensor("tabpk", [SPC, 10, 21, 2528], TAB_DT,
                                kind="ExternalInput").ap(),
        "smpk": nc.dram_tensor("smpk", [P, NBOX * SPC], F32,
                               kind="ExternalInput").ap(),
        "out": nc.dram_tensor("out", [1, 8], F32, kind="ExternalOutput").ap(),
    }
    with tile.TileContext(nc) as tc:
        _build_fast(tc, aps)
    nc.compile()
    _CACHE["fast"] = (nc, None)
    return _CACHE["fast"]


def _kernel_numpy(pred0, pred1, pred2, anchors0, anchors1, anchors2,
                  boxes, labels):
    """Self-contained numpy fallback (only for non-grid anchors)."""
    def softplus(x):
        return np.log1p(np.exp(-np.abs(x))) + np.maximum(x, 0.0)

    tot = np.zeros(5, np.float64)
    for pred, anc in ((pred0, anchors0), (pred1, anchors1),
                      (pred2, anchors2)):
        B, ch, H, W = pred.shape
        p = pred.transpose(0, 2, 3, 1).reshape(B, H * W * 3, 8)
        anc = np.asarray(anc, np.float64)
        aa = (anc[:, 2] - anc[:, 0]) * (anc[:, 3] - anc[:, 1])
        for b in range(B):
            bx = np.asarray(boxes[b], np.float64)
            ab = (bx[:, 2] - bx[:, 0]) * (bx[:, 3] - bx[:, 1])
            lt = np.maximum(anc[:, None, :2], bx[None, :, :2])
            rb = np.minimum(anc[:, None, 2:], bx[None, :, 2:])
            wh = np.clip(rb - lt, 0.0, None)
            inter = wh[..., 0] * wh[..., 1]
            iou = inter / (aa[:, None] + ab[None, :] - inter + 1e-9)
            best = iou.max(1)
            bidx = iou.argmax(1)
            pos = best >= 0.5
            neg = best < 0.3
            x = p[b, :, 4]
            oall = softplus(x) - x * pos
            npos = int(pos.sum())
            k = int(min(neg.sum(), 3 * max(npos, 1)))
            nl = np.where(neg, softplus(x), -1.0)
            order = np.argsort(-nl, kind="stable")
            sel = np.zeros(len(x), bool)
            sel[order[:k]] = True
            sel &= neg
            tot[0] += oall[pos | sel].sum()
            logit = p[b, :, 5:]
            m = logit.max(-1, keepdims=True)
            lse = np.log(np.exp(logit - m).sum(-1)) + m[:, 0]
            tgt = np.clip(labels[b][bidx] - 1, 0, 2)
            ce = lse - np.take_along_axis(logit, tgt[:, None], 1)[:, 0]
            tot[1] += ce[pos].sum()
            mb = bx[bidx]
            aw = anc[:, 2] - anc[:, 0]
            ah = anc[:, 3] - anc[:, 1]
            enc = np.stack([
                (0.5 * (mb[:, 0] + mb[:, 2]) - (anc[:, 0] + 0.5 * aw)) / aw,
                (0.5 * (mb[:, 1] + mb[:, 3]) - (anc[:, 1] + 0.5 * ah)) / ah,
                np.log((mb[:, 2] - mb[:, 0]) / aw),
                np.log((mb[:, 3] - mb[:, 1]) / ah)], -1)
            d = np.abs(p[b, :, :4] - enc)
            sl1 = np.where(d < 1.0, 0.5 * d * d, d - 0.5).sum(-1)
            tot[2] += sl1[pos].sum()
            tot[3] += npos
            tot[4] += int(sel.sum())
    norm = np.float32(max(tot[3], 1.0))
    lo = np.float32(tot[0] / norm)
    lc = np.float32(tot[1] / norm)
    ll = np.float32(tot[2] / norm)
    return (lo, lc, ll, np.float32(lo + lc + 2.0 * ll),
            np.float32(tot[3]), np.float32(tot[4]))


def kernel(pred0, pred1, pred2, anchors0, anchors1, anchors2, boxes, labels,
           _want_results=False, _trace=False):
    static = _host_static([anchors0, anchors1, anchors2])
    if static is None:   # pragma: no cover
        out = _kernel_numpy(pred0, pred1, pred2, anchors0, anchors1,
                            anchors2, boxes, labels)
        out = tuple(np.asarray(v, np.float32) for v in out)
        return (out, None) if _want_results else out
    nc, _ = _get_compiled_fast()
    in_maps = []
    for c in range(NCORES):
        sl = slice(c * SPC, (c + 1) * SPC)
        tabpk, smpk = _host_percore(boxes[sl], labels[sl], static)
        tabpk = tabpk.astype(ml_dtypes.bfloat16)
        in_maps.append({
            "pred0": np.ascontiguousarray(pred0[sl], np.float32),
            "pred1": np.ascontiguousarray(pred1[sl], np.float32),
            "pred2": np.ascontiguousarray(pred2[sl], np.float32),
            "ancpk": static["ancpk"],
            "idm": static["idm"],
            "tabpk": np.ascontiguousarray(tabpk),
            "smpk": np.ascontiguousarray(smpk),
        })
    res = bass_utils.run_bass_kernel_spmd(
        nc, in_maps, core_ids=list(range(NCORES)), trace=_trace)
    parts = np.stack([res.results[c]["out"][0] for c in range(NCORES)])
    tot = parts.sum(axis=0, dtype=np.float64).astype(np.float32)
    tot_obj, tot_cls, tot_loc, tot_pos, tot_neg = tot[:5]
    norm = np.float32(max(tot_pos, np.float32(1.0)))
    lo = np.float32(tot_obj / norm)
    lc = np.float32(tot_cls / norm)
    ll = np.float32(tot_loc / norm)
    ltot = np.float32(lo + lc + np.float32(2.0) * ll)
    out = (lo, lc, ll, ltot, np.float32(tot_pos), np.float32(tot_neg))
    out = tuple(np.asarray(v, np.float32) for v in out)
    if _want_results:
        return out, res
    return out
